# revision 17
# baseline (speedup 1.0000x reference)
"""Multi-head causal attention (QKV proj + RoPE + softmax) on 8 TRN2 NeuronCores.

Sharding: batch 4-way x head-group 2-way -> each core handles 1 batch and 8
contiguous heads (512 output channels). No collectives; host gathers slices.

v2 schedule: the projections are no longer a serial prelude. Attention units
start as soon as qh[m0, 0:512], kh[m0, 0:512] and vsb[st0] exist, and the
remaining projection work is drip-fed into the PE queue between unit k-tiles,
filling PE slack in the Act-gated softmax pipeline (exp of [128,1024] =
~1.1us vs ~0.65us PE per k-tile). The unit stream is software-pipelined one
k-tile deep (scores of step i+1 issue before PV of step i) so the in-order
PE queue never head-blocks on an exp. x tiles are DMA'd quarter-wise in
consumption order via prefetch pseudo-chains (bounded SBUF pool, no Sync
queue deadlock); RoPE swaps, output DMAs and causal-mask multiplies run on
the otherwise idle GpSimd queue.

Per-core algorithm (matmul compute in bf16, fp32 PSUM accumulation):
  - host passes x.T and W.T shards so matmuls contract over the partition dim.
  - q/k weights are row-permuted per head into [even|odd] so RoPE becomes
    rot = x*cs + swap32(x)*sn (swap32 = SBUF partition-block swap by DMA).
  - scores are computed transposed, S_T[k, q] = kh_T.T @ qh_T, with the A/B
    heads of a 128-row tile on PE row groups 0/1 running concurrently.
  - causal diagonal tiles are NARROWED: scores/exp/PV touch only the
    unmasked q-range (q >= 128*p within the tile); the remaining 128-wide
    triangle is masked by one GpSimd multiply.
  - attnT[d, q] = sum_kt V_tile[k,d|1].T @ P_T[k, q]; a ones-column in V
    makes row 64 the softmax denominator. Division + transpose + v-bias on
    host.
"""

import sys
import types

import numpy as np
import ml_dtypes

BF16 = ml_dtypes.bfloat16
SEQ, EMB, NHEADS, BATCH = 2048, 1024, 16, 4
HD, HALF = 64, 32
HPC = 8          # heads per core
DH = 512         # output dims per core
NE = EMB // 128  # 8 contraction tiles
NT = 4           # head-pair (128-row) dout tiles
NKT = SEQ // 128  # 16 key tiles
NQC = SEQ // 512  # 4 query chunks


def _install_ntff_shim():
    """The image's antenv lacks axon_hooks; synthesize it from trn_agent_boot
    so run_bass_kernel_spmd(trace=True) can profile. Harmless if unused."""
    try:
        import antenv.axon_hooks  # noqa: F401
        return
    except ImportError:
        pass
    try:
        from trn_agent_boot.trn_boot import _ntff_profile_via_ctypes
        import antenv
    except ImportError:
        return
    hook = _ntff_profile_via_ctypes("/opt/axon/libaxon_pjrt.so")
    mod = types.ModuleType("antenv.axon_hooks")
    mod.get_axon_ntff_profile_hook = lambda: hook
    mod.set_axon_ntff_profile_hook = lambda h: None
    sys.modules["antenv.axon_hooks"] = mod
    antenv.axon_hooks = mod


_built = {}


def build(causal=True):
    if causal in _built:
        return _built[causal]
    import concourse.mybir as mybir
    import concourse.tile as tile
    from concourse import bacc

    f32 = mybir.dt.float32
    bf = mybir.dt.bfloat16
    EXP = mybir.ActivationFunctionType.Exp
    MUL = mybir.AluOpType.mult
    ADD = mybir.AluOpType.add

    nc = bacc.Bacc(None, target_bir_lowering=False, debug=False)
    with tile.TileContext(nc) as tc:
        with tc.tile_pool(name="dram", bufs=1, space="DRAM") as dram:
            xq_d = dram.tile([EMB, SEQ], bf, kind="ExternalInput", name="xq", uniquify=False)
            xk_d = dram.tile([EMB, SEQ], bf, kind="ExternalInput", name="xk", uniquify=False)
            xv_d = dram.tile([EMB, SEQ], bf, kind="ExternalInput", name="xv", uniquify=False)
            wq_d = dram.tile([EMB, DH], bf, kind="ExternalInput", name="wq", uniquify=False)
            wk_d = dram.tile([EMB, DH], bf, kind="ExternalInput", name="wk", uniquify=False)
            wv_d = dram.tile([EMB, DH], bf, kind="ExternalInput", name="wv", uniquify=False)
            bqc_d = dram.tile([128, NT], f32, kind="ExternalInput", name="bqc", uniquify=False)
            bkc_d = dram.tile([128, NT], f32, kind="ExternalInput", name="bkc", uniquify=False)
            cs_d = dram.tile([128, SEQ], bf, kind="ExternalInput", name="cs2", uniquify=False)
            sn_d = dram.tile([128, SEQ], bf, kind="ExternalInput", name="sn2", uniquify=False)
            mk_d = dram.tile([128, 256], bf, kind="ExternalInput", name="msk", uniquify=False)
            outT_d = dram.tile([DH, SEQ], f32, kind="ExternalOutput", name="outT", uniquify=False)
            l_d = dram.tile([HPC, SEQ], f32, kind="ExternalOutput", name="lsum", uniquify=False)

            with tc.tile_pool(name="const", bufs=1) as cp, \
                 tc.tile_pool(name="xin", bufs=52) as xp, \
                 tc.tile_pool(name="rope", bufs=3) as rp, \
                 tc.tile_pool(name="ostage", bufs=4) as op, \
                 tc.tile_pool(name="pp", bufs=2, space="PSUM") as pp, \
                 tc.tile_pool(name="sp", bufs=2, space="PSUM") as sp, \
                 tc.tile_pool(name="tA", bufs=1, space="PSUM") as ptA, \
                 tc.tile_pool(name="tB", bufs=1, space="PSUM") as ptB:

                qh = cp.tile([128, NT, SEQ], bf, name="qh")
                kh = cp.tile([128, NT, SEQ], bf, name="kh")
                vsb = cp.tile([128, NKT, HPC * 65], bf, name="vsb")
                probs = cp.tile([128, 2, NKT, 512], bf, name="probs")
                w_sb = {n: cp.tile([128, NE, DH], bf, name=f"w_{n}") for n in "qkv"}
                b_sb = {n: cp.tile([128, NT], f32, name=f"b_{n}") for n in "qk"}
                cs = cp.tile([128, SEQ], bf, name="cs")
                sn = cp.tile([128, SEQ], bf, name="sn")
                msk = cp.tile([128, 2, 128], bf, name="mskt")

                # small constants first
                nc.sync.dma_start(out=b_sb["q"][:, :], in_=bqc_d[:, :])
                nc.sync.dma_start(out=b_sb["k"][:, :], in_=bkc_d[:, :])
                nc.sync.dma_start(out=cs[:, :], in_=cs_d[:, :])
                nc.sync.dma_start(out=sn[:, :], in_=sn_d[:, :])
                nc.sync.dma_start(out=msk[:, :, :],
                                  in_=mk_d[:, :].rearrange("p (h u) -> p h u", h=2))
                nc.vector.memset(
                    vsb[:, :, :].rearrange("p k (h u) -> p k h u", u=65)[:, :, :, 64:65],
                    1.0)

                xt = {}      # (nm, c4) -> [128, NE, 512] SBUF tile
                wd = {"q": wq_d, "k": wk_d, "v": wv_d}
                xd = {"q": xq_d, "k": xk_d, "v": xv_d}

                def chain_dma(nm, c4):
                    """Prefetch pseudo-chain: stage one x quarter (and the
                    weights, on the first quarter)."""
                    for e in range(NE):
                        if c4 == 0:
                            nc.sync.dma_start(out=w_sb[nm][:, e, :],
                                              in_=wd[nm][e * 128:(e + 1) * 128, :])
                        t = xp.tile([128, 512], bf, tag="x", name=f"x{nm}{e}_{c4}")
                        nc.sync.dma_start(
                            out=t[:, :],
                            in_=xd[nm][e * 128:(e + 1) * 128,
                                       c4 * 512:(c4 + 1) * 512])
                        xt[(nm, e, c4)] = t
                    return
                    yield  # pragma: no cover - make this a generator

                # ---- projection chains (generators; each yield ~= 2 e-pairs
                # of PE work). qk chain: one (m, q-chunk) -> psum -> bias-add
                # eviction -> partition-swap DMA -> RoPE into qh/kh. ----
                def chain_qk(nm, m, c):
                    dst = qh if nm == "q" else kh
                    ps = pp.tile([128, 512], f32, tag="p", name=f"pp{nm}{m}{c}")
                    for e in range(NE):
                        nc.tensor.matmul(
                            ps[0:64, :],
                            w_sb[nm][:, e, m * 128:m * 128 + 64],
                            xt[(nm, e, c)][:, :],
                            start=(e == 0), stop=(e == NE - 1),
                            skip_group_check=True)
                        nc.tensor.matmul(
                            ps[64:128, :],
                            w_sb[nm][:, e, m * 128 + 64:(m + 1) * 128],
                            xt[(nm, e, c)][:, :],
                            start=(e == 0), stop=(e == NE - 1),
                            skip_group_check=True)
                        if e % 2 == 1 and e < NE - 1:
                            yield
                    tmp = rp.tile([128, 512], bf, tag="tmp", bufs=6, name=f"t{nm}{m}{c}")
                    nc.vector.tensor_scalar_add(tmp[:, :], ps[:, :],
                                                b_sb[nm][:, m:m + 1])
                    tsw = rp.tile([128, 512], bf, tag="tsw", bufs=4, name=f"w{nm}{m}{c}")
                    for blk in range(4):
                        s = blk ^ 1
                        nc.sync.dma_start(out=tsw[blk * 32:(blk + 1) * 32, :],
                                          in_=tmp[s * 32:(s + 1) * 32, :])
                    yield
                    cc = slice(c * 512, (c + 1) * 512)
                    m2 = rp.tile([128, 512], bf, tag="m2", bufs=4, name=f"m{nm}{m}{c}")
                    nc.vector.tensor_tensor(dst[:, m, cc], tmp[:, :], cs[:, cc], MUL)
                    nc.vector.tensor_tensor(m2[:, :], tsw[:, :], sn[:, cc], MUL)
                    nc.vector.tensor_tensor(dst[:, m, cc], dst[:, m, cc], m2[:, :], ADD)

                def chain_v(st):
                    c4, o = st // 4, (st % 4) * 128
                    ps = pp.tile([128, 512], f32, tag="p", name=f"ppv{st}")
                    for e in range(NE):
                        nc.tensor.matmul(
                            ps[0:64, :],
                            xt[("v", e, c4)][:, o:o + 64],
                            w_sb["v"][:, e, :],
                            start=(e == 0), stop=(e == NE - 1),
                            skip_group_check=True)
                        nc.tensor.matmul(
                            ps[64:128, :],
                            xt[("v", e, c4)][:, o + 64:o + 128],
                            w_sb["v"][:, e, :],
                            start=(e == 0), stop=(e == NE - 1),
                            skip_group_check=True)
                        if e % 2 == 1 and e < NE - 1:
                            yield
                    nc.vector.tensor_copy(
                        vsb[:, st, :]
                        .rearrange("p (h u) -> p h u", u=65)[:, :, 0:64],
                        ps[:, :].rearrange("p (h d) -> p h d", d=64))

                # unit emission order and the matching chain priority order
                unit_order = [(0, 0), (1, 0), (0, 1), (2, 0), (1, 1), (3, 0),
                              (2, 1), (0, 2), (3, 1), (1, 2), (2, 2), (0, 3),
                              (3, 2), (1, 3), (2, 3), (3, 3)]
                spec_list = [
                    ("D", "q", 0), ("Q", 0, 0), ("D", "k", 0), ("K", 0, 0),
                    ("D", "v", 0), ("V", 0), ("V", 1), ("V", 2), ("V", 3),
                    ("Q", 1, 0), ("K", 1, 0),
                    ("D", "q", 1), ("Q", 0, 1), ("D", "k", 1), ("K", 0, 1),
                    ("D", "v", 1), ("V", 4), ("V", 5), ("V", 6), ("V", 7),
                    ("Q", 2, 0), ("K", 2, 0), ("Q", 1, 1), ("K", 1, 1),
                    ("Q", 3, 0), ("K", 3, 0), ("Q", 2, 1), ("K", 2, 1),
                    ("D", "q", 2), ("Q", 0, 2), ("D", "k", 2), ("K", 0, 2),
                    ("D", "v", 2), ("V", 8), ("V", 9), ("V", 10), ("V", 11),
                    ("Q", 3, 1), ("K", 3, 1), ("Q", 1, 2), ("K", 1, 2),
                    ("Q", 2, 2), ("K", 2, 2),
                    ("D", "q", 3), ("Q", 0, 3), ("D", "k", 3), ("K", 0, 3),
                    ("D", "v", 3), ("V", 12), ("V", 13), ("V", 14), ("V", 15),
                    ("Q", 3, 2), ("K", 3, 2), ("Q", 1, 3), ("K", 1, 3),
                    ("Q", 2, 3), ("K", 2, 3), ("Q", 3, 3), ("K", 3, 3),
                ]
                gens = []
                ord_of = {}
                for spec in spec_list:
                    ord_of[spec] = len(gens)
                    if spec[0] == "D":
                        gens.append(chain_dma(spec[1], spec[2]))
                    elif spec[0] == "V":
                        gens.append(chain_v(spec[1]))
                    else:
                        gens.append(chain_qk(spec[0].lower(), spec[1], spec[2]))
                state = {"pos": 0}

                def drive(n):
                    while n > 0 and state["pos"] < len(gens):
                        try:
                            next(gens[state["pos"]])
                            n -= 1
                        except StopIteration:
                            state["pos"] += 1

                def require(spec):
                    """Fully emit every chain up to and including `spec` —
                    Tile dependencies are tracked in EMISSION order, so a
                    consumer must never be emitted before its producer."""
                    target = ord_of[spec] + 1
                    while state["pos"] < target:
                        try:
                            next(gens[state["pos"]])
                        except StopIteration:
                            state["pos"] += 1

                # ---- flat, 1-deep software-pipelined unit stream ----
                def nkt_of(j):
                    return 4 * (j + 1) if causal else NKT

                seq_steps = [(t, j, kt) for (t, j) in unit_order
                             for kt in range(nkt_of(j))]
                pt_cur = {}
                ps_of = {}

                def emit_scores_exp(t, j, kt):
                    require(("Q", t, j))
                    require(("K", t, kt // 4))
                    p = kt - 4 * j
                    q0 = 128 * p if (causal and p > 0) else 0
                    ps = sp.tile([128, 1024], f32, tag="s", name=f"ps{t}{j}_{kt}")
                    ps_of[(t, j, kt)] = (ps, q0)
                    for half in (0, 1):
                        po = half * 64
                        nc.tensor.matmul(
                            ps[:, half * 512 + q0:(half + 1) * 512],
                            kh[po:po + 64, t, kt * 128:(kt + 1) * 128],
                            qh[po:po + 64, t, j * 512 + q0:(j + 1) * 512],
                            start=True, stop=True, skip_group_check=True)
                    nc.scalar.activation(
                        probs[:, :, kt, q0:512],
                        ps[:, :].rearrange("p (h u) -> p h u", h=2)[:, :, q0:512],
                        EXP)
                    if causal and p >= 0:
                        nc.gpsimd.tensor_tensor(
                            probs[:, :, kt, q0:q0 + 128],
                            probs[:, :, kt, q0:q0 + 128],
                            msk[:, :, :], MUL)

                def emit_pv(t, j, kt):
                    require(("V", kt))
                    nkt = nkt_of(j)
                    if kt == 0:
                        pt_cur[0] = ptA.tile([65, 512], f32, tag="t0", name=f"pt0_{t}{j}")
                        pt_cur[1] = ptB.tile([65, 512], f32, tag="t1", name=f"pt1_{t}{j}")
                    _, q0 = ps_of.pop((t, j, kt))
                    for half in (0, 1):
                        lh = 2 * t + half
                        nc.tensor.matmul(
                            pt_cur[half][:, q0:512],
                            vsb[:, kt, lh * 65:(lh + 1) * 65],
                            probs[:, half, kt, q0:512],
                            start=(kt == 0), stop=(kt == nkt - 1),
                            skip_group_check=True)
                    if kt == nkt - 1:
                        for half in (0, 1):
                            lh = 2 * t + half
                            ost = op.tile([65, 512], f32, tag="ost",
                                          name=f"os{half}_{t}{j}")
                            nc.vector.tensor_copy(ost[:, :], pt_cur[half][:, :])
                            nc.sync.dma_start(
                                out=outT_d[lh * 64:(lh + 1) * 64,
                                           j * 512:(j + 1) * 512],
                                in_=ost[0:64, :])
                            nc.sync.dma_start(
                                out=l_d[lh:lh + 1, j * 512:(j + 1) * 512],
                                in_=ost[64:65, :])

                # prelude: D(q0), Q00, D(k0), K00, D(v0), V0
                require(("V", 0))
                n = len(seq_steps)
                for i in range(n + 1):
                    if i < n:
                        emit_scores_exp(*seq_steps[i])
                        drive(1)
                    if i > 0:
                        emit_pv(*seq_steps[i - 1])
                        drive(1)
                while state["pos"] < len(gens):
                    drive(8)
    _built[causal] = nc
    nc.compile()
    return nc


def _prep_core_inputs(c, q, k, v, Wq, bq, Wk, bk, Wv, bv, sin, cos):
    b, hh = c // 2, c % 2
    hs = slice(hh * DH, (hh + 1) * DH)

    perm = np.empty(DH, np.int64)
    for lh in range(HPC):
        base = (hh * HPC + lh) * HD
        perm[lh * HD:lh * HD + HALF] = base + 2 * np.arange(HALF)
        perm[lh * HD + HALF:(lh + 1) * HD] = base + 2 * np.arange(HALF) + 1

    s = 0.125  # 1/sqrt(HD), folded into the q projection
    wq = np.ascontiguousarray((Wq[perm, :] * s).T).astype(BF16)
    wk = np.ascontiguousarray(Wk[perm, :].T).astype(BF16)
    wv = np.ascontiguousarray(Wv[hs, :].T).astype(BF16)

    p32 = np.arange(128) % 32
    cs2 = cos[:, p32].T.astype(BF16)
    sgn = np.where((np.arange(128) // 32) % 2 == 0, -1.0, 1.0).astype(np.float32)
    sn2 = (sin[:, p32] * sgn[None, :]).T.astype(BF16)

    kk = np.arange(128)[:, None]
    qq = np.arange(128)[None, :]
    tri = (kk <= qq)  # [128, 128] triangle for the diagonal tiles
    msk = np.concatenate([tri, tri], axis=1).astype(BF16)  # [128, 256]

    return {
        "xq": np.ascontiguousarray(q[b].T).astype(BF16),
        "xk": np.ascontiguousarray(k[b].T).astype(BF16),
        "xv": np.ascontiguousarray(v[b].T).astype(BF16),
        "wq": wq, "wk": wk, "wv": wv,
        "bqc": np.ascontiguousarray((bq[perm] * s).reshape(NT, 128).T, np.float32),
        "bkc": np.ascontiguousarray(bk[perm].reshape(NT, 128).T, np.float32),
        "cs2": cs2, "sn2": sn2, "msk": msk,
    }


def prep_in_maps(q, k, v, Wq, bq, Wk, bk, Wv, bv, sin, cos):
    args = [np.asarray(a, np.float32) for a in (q, k, v, Wq, bq, Wk, bk, Wv, bv, sin, cos)]
    maps = [_prep_core_inputs(c, *args) for c in range(8)]
    return maps, args[8]  # bv needed on host in assemble()


def assemble(results, bv):
    out = np.empty((BATCH, SEQ, EMB), np.float32)
    for c in range(8):
        b, hh = c // 2, c % 2
        outT = np.asarray(results[c]["outT"], np.float32)
        l = np.asarray(results[c]["lsum"], np.float32)
        a = outT.reshape(HPC, HD, SEQ) / l[:, None, :]
        out[b, :, hh * DH:(hh + 1) * DH] = a.reshape(DH, SEQ).T \
            + bv[hh * DH:(hh + 1) * DH][None, :]
    return out


def run(in_maps, causal=True, trace=False, **kw):
    _install_ntff_shim()
    from concourse.bass_utils import run_bass_kernel_spmd
    nc = build(causal)
    return run_bass_kernel_spmd(nc, in_maps, core_ids=list(range(8)), trace=trace, **kw)


def kernel(q, k, v, Wq, bq, Wk, bk, Wv, bv, sin, cos, mask):
    in_maps, bv_f = prep_in_maps(q, k, v, Wq, bq, Wk, bk, Wv, bv, sin, cos)
    r = run(in_maps, causal=bool(mask))
    return assemble(r.results, bv_f)


# revision 19
# speedup vs baseline: 1.0091x; 1.0091x over previous
"""Multi-head causal attention (QKV proj + RoPE + softmax) on 8 TRN2 NeuronCores.

Sharding: batch 4-way x head-group 2-way -> each core handles 1 batch and 8
contiguous heads (512 output channels). No collectives; host gathers slices.

v2 schedule: the projections are no longer a serial prelude. Attention units
start as soon as qh[m0, 0:512], kh[m0, 0:512] and vsb[st0] exist, and the
remaining projection work is drip-fed into the PE queue between unit k-tiles,
filling PE slack in the Act-gated softmax pipeline (exp of [128,1024] =
~1.1us vs ~0.65us PE per k-tile). The unit stream is software-pipelined one
k-tile deep (scores of step i+1 issue before PV of step i) so the in-order
PE queue never head-blocks on an exp. x tiles are DMA'd quarter-wise in
consumption order via prefetch pseudo-chains (bounded SBUF pool, no Sync
queue deadlock); RoPE swaps, output DMAs and causal-mask multiplies run on
the otherwise idle GpSimd queue.

Per-core algorithm (matmul compute in bf16, fp32 PSUM accumulation):
  - host passes x.T and W.T shards so matmuls contract over the partition dim.
  - q/k weights are row-permuted per head into [even|odd] so RoPE becomes
    rot = x*cs + swap32(x)*sn (swap32 = SBUF partition-block swap by DMA).
  - scores are computed transposed, S_T[k, q] = kh_T.T @ qh_T, with the A/B
    heads of a 128-row tile on PE row groups 0/1 running concurrently.
  - causal diagonal tiles are NARROWED: scores/exp/PV touch only the
    unmasked q-range (q >= 128*p within the tile); the remaining 128-wide
    triangle is masked by one GpSimd multiply.
  - attnT[d, q] = sum_kt V_tile[k,d|1].T @ P_T[k, q]; a ones-column in V
    makes row 64 the softmax denominator. Division + transpose + v-bias on
    host.
"""

import sys
import types

import numpy as np
import ml_dtypes

BF16 = ml_dtypes.bfloat16
SEQ, EMB, NHEADS, BATCH = 2048, 1024, 16, 4
HD, HALF = 64, 32
HPC = 8          # heads per core
DH = 512         # output dims per core
NE = EMB // 128  # 8 contraction tiles
NT = 4           # head-pair (128-row) dout tiles
NKT = SEQ // 128  # 16 key tiles
NQC = SEQ // 512  # 4 query chunks


def _install_ntff_shim():
    """The image's antenv lacks axon_hooks; synthesize it from trn_agent_boot
    so run_bass_kernel_spmd(trace=True) can profile. Harmless if unused."""
    try:
        import antenv.axon_hooks  # noqa: F401
        return
    except ImportError:
        pass
    try:
        from trn_agent_boot.trn_boot import _ntff_profile_via_ctypes
        import antenv
    except ImportError:
        return
    hook = _ntff_profile_via_ctypes("/opt/axon/libaxon_pjrt.so")
    mod = types.ModuleType("antenv.axon_hooks")
    mod.get_axon_ntff_profile_hook = lambda: hook
    mod.set_axon_ntff_profile_hook = lambda h: None
    sys.modules["antenv.axon_hooks"] = mod
    antenv.axon_hooks = mod


_built = {}


def build(causal=True):
    if causal in _built:
        return _built[causal]
    import concourse.mybir as mybir
    import concourse.tile as tile
    from concourse import bacc

    f32 = mybir.dt.float32
    bf = mybir.dt.bfloat16
    EXP = mybir.ActivationFunctionType.Exp
    MUL = mybir.AluOpType.mult
    ADD = mybir.AluOpType.add

    nc = bacc.Bacc(None, target_bir_lowering=False, debug=False)
    with tile.TileContext(nc) as tc:
        with tc.tile_pool(name="dram", bufs=1, space="DRAM") as dram:
            xq_d = dram.tile([EMB, SEQ], bf, kind="ExternalInput", name="xq", uniquify=False)
            xk_d = dram.tile([EMB, SEQ], bf, kind="ExternalInput", name="xk", uniquify=False)
            xv_d = dram.tile([EMB, SEQ], bf, kind="ExternalInput", name="xv", uniquify=False)
            wq_d = dram.tile([EMB, DH], bf, kind="ExternalInput", name="wq", uniquify=False)
            wk_d = dram.tile([EMB, DH], bf, kind="ExternalInput", name="wk", uniquify=False)
            wv_d = dram.tile([EMB, DH], bf, kind="ExternalInput", name="wv", uniquify=False)
            bqc_d = dram.tile([128, NT], f32, kind="ExternalInput", name="bqc", uniquify=False)
            bkc_d = dram.tile([128, NT], f32, kind="ExternalInput", name="bkc", uniquify=False)
            cs_d = dram.tile([128, SEQ], bf, kind="ExternalInput", name="cs2", uniquify=False)
            sn_d = dram.tile([128, SEQ], bf, kind="ExternalInput", name="sn2", uniquify=False)
            mk_d = dram.tile([128, 256], bf, kind="ExternalInput", name="msk", uniquify=False)
            outT_d = dram.tile([DH, SEQ], f32, kind="ExternalOutput", name="outT", uniquify=False)
            l_d = dram.tile([HPC, SEQ], f32, kind="ExternalOutput", name="lsum", uniquify=False)

            with tc.tile_pool(name="const", bufs=1) as cp, \
                 tc.tile_pool(name="xin", bufs=8) as xp, \
                 tc.tile_pool(name="rope", bufs=3) as rp, \
                 tc.tile_pool(name="ostage", bufs=4) as op, \
                 tc.tile_pool(name="pp", bufs=2, space="PSUM") as pp, \
                 tc.tile_pool(name="sp", bufs=2, space="PSUM") as sp, \
                 tc.tile_pool(name="tA", bufs=1, space="PSUM") as ptA, \
                 tc.tile_pool(name="tB", bufs=1, space="PSUM") as ptB:

                qh = cp.tile([128, NT, SEQ], bf, name="qh")
                kh = cp.tile([128, NT, SEQ], bf, name="kh")
                vsb = cp.tile([128, NKT, HPC * 65], bf, name="vsb")
                probs = cp.tile([128, 2, NKT, 512], bf, name="probs")
                w_sb = {n: cp.tile([128, NE, DH], bf, name=f"w_{n}") for n in "qkv"}
                b_sb = {n: cp.tile([128, NT], f32, name=f"b_{n}") for n in "qk"}
                cs = cp.tile([128, SEQ], bf, name="cs")
                sn = cp.tile([128, SEQ], bf, name="sn")
                msk = cp.tile([128, 2, 128], bf, name="mskt")

                # small constants first
                nc.sync.dma_start(out=b_sb["q"][:, :], in_=bqc_d[:, :])
                nc.sync.dma_start(out=b_sb["k"][:, :], in_=bkc_d[:, :])
                nc.sync.dma_start(out=cs[:, :], in_=cs_d[:, :])
                nc.sync.dma_start(out=sn[:, :], in_=sn_d[:, :])
                nc.sync.dma_start(out=msk[:, :, :],
                                  in_=mk_d[:, :].rearrange("p (h u) -> p h u", h=2))
                nc.vector.memset(
                    vsb[:, :, :].rearrange("p k (h u) -> p k h u", u=65)[:, :, :, 64:65],
                    1.0)

                xt = {}      # (nm, c4) -> [128, NE, 512] SBUF tile
                wd = {"q": wq_d, "k": wk_d, "v": wv_d}
                xd = {"q": xq_d, "k": xk_d, "v": xv_d}

                def chain_dma(nm, c4):
                    """Prefetch pseudo-chain: one 3D DMA stages the whole x
                    quarter (+ the weight matrix, on the first quarter)."""
                    if c4 == 0:
                        nc.sync.dma_start(
                            out=w_sb[nm][:, :, :],
                            in_=wd[nm][:, :].rearrange("(e p) d -> p e d", p=128))
                    t = xp.tile([128, NE, 512], bf, tag="x", name=f"x{nm}_{c4}")
                    nc.sync.dma_start(
                        out=t[:, :, :],
                        in_=xd[nm][:, c4 * 512:(c4 + 1) * 512]
                        .rearrange("(e p) s -> p e s", p=128))
                    xt[(nm, c4)] = t
                    return
                    yield  # pragma: no cover - make this a generator

                # ---- projection chains (generators; each yield ~= 2 e-pairs
                # of PE work). qk chain: one (m, q-chunk) -> psum -> bias-add
                # eviction -> partition-swap DMA -> RoPE into qh/kh. ----
                def chain_qk(nm, m, c):
                    dst = qh if nm == "q" else kh
                    ps = pp.tile([128, 512], f32, tag="p", name=f"pp{nm}{m}{c}")
                    for e in range(NE):
                        nc.tensor.matmul(
                            ps[0:64, :],
                            w_sb[nm][:, e, m * 128:m * 128 + 64],
                            xt[(nm, c)][:, e, :],
                            start=(e == 0), stop=(e == NE - 1),
                            skip_group_check=True)
                        nc.tensor.matmul(
                            ps[64:128, :],
                            w_sb[nm][:, e, m * 128 + 64:(m + 1) * 128],
                            xt[(nm, c)][:, e, :],
                            start=(e == 0), stop=(e == NE - 1),
                            skip_group_check=True)
                        if e % 2 == 1 and e < NE - 1:
                            yield
                    tmp = rp.tile([128, 512], bf, tag="tmp", bufs=6, name=f"t{nm}{m}{c}")
                    nc.vector.tensor_scalar_add(tmp[:, :], ps[:, :],
                                                b_sb[nm][:, m:m + 1])
                    tsw = rp.tile([128, 512], bf, tag="tsw", bufs=4, name=f"w{nm}{m}{c}")
                    for blk in range(4):
                        s = blk ^ 1
                        nc.gpsimd.dma_start(out=tsw[blk * 32:(blk + 1) * 32, :],
                                            in_=tmp[s * 32:(s + 1) * 32, :])
                    yield
                    cc = slice(c * 512, (c + 1) * 512)
                    m2 = rp.tile([128, 512], bf, tag="m2", bufs=4, name=f"m{nm}{m}{c}")
                    nc.vector.tensor_tensor(dst[:, m, cc], tmp[:, :], cs[:, cc], MUL)
                    nc.vector.tensor_tensor(m2[:, :], tsw[:, :], sn[:, cc], MUL)
                    nc.vector.tensor_tensor(dst[:, m, cc], dst[:, m, cc], m2[:, :], ADD)

                def chain_v(st):
                    c4, o = st // 4, (st % 4) * 128
                    ps = pp.tile([128, 512], f32, tag="p", name=f"ppv{st}")
                    for e in range(NE):
                        nc.tensor.matmul(
                            ps[0:64, :],
                            xt[("v", c4)][:, e, o:o + 64],
                            w_sb["v"][:, e, :],
                            start=(e == 0), stop=(e == NE - 1),
                            skip_group_check=True)
                        nc.tensor.matmul(
                            ps[64:128, :],
                            xt[("v", c4)][:, e, o + 64:o + 128],
                            w_sb["v"][:, e, :],
                            start=(e == 0), stop=(e == NE - 1),
                            skip_group_check=True)
                        if e % 2 == 1 and e < NE - 1:
                            yield
                    nc.vector.tensor_copy(
                        vsb[:, st, :]
                        .rearrange("p (h u) -> p h u", u=65)[:, :, 0:64],
                        ps[:, :].rearrange("p (h d) -> p h d", d=64))

                # unit emission order and the matching chain priority order
                unit_order = [(0, 0), (1, 0), (0, 1), (2, 0), (1, 1), (3, 0),
                              (2, 1), (0, 2), (3, 1), (1, 2), (2, 2), (0, 3),
                              (3, 2), (1, 3), (2, 3), (3, 3)]
                spec_list = [
                    ("D", "q", 0), ("Q", 0, 0), ("D", "k", 0), ("K", 0, 0),
                    ("D", "v", 0), ("V", 0), ("V", 1), ("V", 2), ("V", 3),
                    ("Q", 1, 0), ("K", 1, 0),
                    ("D", "q", 1), ("Q", 0, 1), ("D", "k", 1), ("K", 0, 1),
                    ("D", "v", 1), ("V", 4), ("V", 5), ("V", 6), ("V", 7),
                    ("Q", 2, 0), ("K", 2, 0), ("Q", 1, 1), ("K", 1, 1),
                    ("Q", 3, 0), ("K", 3, 0), ("Q", 2, 1), ("K", 2, 1),
                    ("D", "q", 2), ("Q", 0, 2), ("D", "k", 2), ("K", 0, 2),
                    ("D", "v", 2), ("V", 8), ("V", 9), ("V", 10), ("V", 11),
                    ("Q", 3, 1), ("K", 3, 1), ("Q", 1, 2), ("K", 1, 2),
                    ("Q", 2, 2), ("K", 2, 2),
                    ("D", "q", 3), ("Q", 0, 3), ("D", "k", 3), ("K", 0, 3),
                    ("D", "v", 3), ("V", 12), ("V", 13), ("V", 14), ("V", 15),
                    ("Q", 3, 2), ("K", 3, 2), ("Q", 1, 3), ("K", 1, 3),
                    ("Q", 2, 3), ("K", 2, 3), ("Q", 3, 3), ("K", 3, 3),
                ]
                gens = []
                ord_of = {}
                for spec in spec_list:
                    ord_of[spec] = len(gens)
                    if spec[0] == "D":
                        gens.append(chain_dma(spec[1], spec[2]))
                    elif spec[0] == "V":
                        gens.append(chain_v(spec[1]))
                    else:
                        gens.append(chain_qk(spec[0].lower(), spec[1], spec[2]))
                state = {"pos": 0}

                def drive(n):
                    while n > 0 and state["pos"] < len(gens):
                        try:
                            next(gens[state["pos"]])
                            n -= 1
                        except StopIteration:
                            state["pos"] += 1

                def require(spec):
                    """Fully emit every chain up to and including `spec` —
                    Tile dependencies are tracked in EMISSION order, so a
                    consumer must never be emitted before its producer."""
                    target = ord_of[spec] + 1
                    while state["pos"] < target:
                        try:
                            next(gens[state["pos"]])
                        except StopIteration:
                            state["pos"] += 1

                # ---- flat, 1-deep software-pipelined unit stream ----
                def nkt_of(j):
                    return 4 * (j + 1) if causal else NKT

                seq_steps = [(t, j, kt) for (t, j) in unit_order
                             for kt in range(nkt_of(j))]
                pt_cur = {}
                ps_of = {}

                def emit_scores_exp(t, j, kt):
                    require(("Q", t, j))
                    require(("K", t, kt // 4))
                    p = kt - 4 * j
                    q0 = 128 * p if (causal and p > 0) else 0
                    ps = sp.tile([128, 1024], f32, tag="s", name=f"ps{t}{j}_{kt}")
                    ps_of[(t, j, kt)] = (ps, q0)
                    for half in (0, 1):
                        po = half * 64
                        nc.tensor.matmul(
                            ps[:, half * 512 + q0:(half + 1) * 512],
                            kh[po:po + 64, t, kt * 128:(kt + 1) * 128],
                            qh[po:po + 64, t, j * 512 + q0:(j + 1) * 512],
                            start=True, stop=True, skip_group_check=True)
                    nc.scalar.activation(
                        probs[:, :, kt, q0:512],
                        ps[:, :].rearrange("p (h u) -> p h u", h=2)[:, :, q0:512],
                        EXP)
                    if causal and p >= 0:
                        nc.gpsimd.tensor_tensor(
                            probs[:, :, kt, q0:q0 + 128],
                            probs[:, :, kt, q0:q0 + 128],
                            msk[:, :, :], MUL)

                def emit_pv(t, j, kt):
                    require(("V", kt))
                    nkt = nkt_of(j)
                    if kt == 0:
                        pt_cur[0] = ptA.tile([65, 512], f32, tag="t0", name=f"pt0_{t}{j}")
                        pt_cur[1] = ptB.tile([65, 512], f32, tag="t1", name=f"pt1_{t}{j}")
                    _, q0 = ps_of.pop((t, j, kt))
                    for half in (0, 1):
                        lh = 2 * t + half
                        nc.tensor.matmul(
                            pt_cur[half][:, q0:512],
                            vsb[:, kt, lh * 65:(lh + 1) * 65],
                            probs[:, half, kt, q0:512],
                            start=(kt == 0), stop=(kt == nkt - 1),
                            skip_group_check=True)
                    if kt == nkt - 1:
                        for half in (0, 1):
                            lh = 2 * t + half
                            ost = op.tile([65, 512], f32, tag="ost",
                                          name=f"os{half}_{t}{j}")
                            nc.vector.tensor_copy(ost[:, :], pt_cur[half][:, :])
                            nc.sync.dma_start(
                                out=outT_d[lh * 64:(lh + 1) * 64,
                                           j * 512:(j + 1) * 512],
                                in_=ost[0:64, :])
                            nc.sync.dma_start(
                                out=l_d[lh:lh + 1, j * 512:(j + 1) * 512],
                                in_=ost[64:65, :])

                # prelude: D(q0), Q00, D(k0), K00, D(v0), V0
                require(("V", 0))
                n = len(seq_steps)
                for i in range(n + 1):
                    if i < n:
                        emit_scores_exp(*seq_steps[i])
                        drive(1)
                    if i > 0:
                        emit_pv(*seq_steps[i - 1])
                        drive(1)
                while state["pos"] < len(gens):
                    drive(8)
    _built[causal] = nc
    nc.compile()
    return nc


def _prep_core_inputs(c, q, k, v, Wq, bq, Wk, bk, Wv, bv, sin, cos):
    b, hh = c // 2, c % 2
    hs = slice(hh * DH, (hh + 1) * DH)

    perm = np.empty(DH, np.int64)
    for lh in range(HPC):
        base = (hh * HPC + lh) * HD
        perm[lh * HD:lh * HD + HALF] = base + 2 * np.arange(HALF)
        perm[lh * HD + HALF:(lh + 1) * HD] = base + 2 * np.arange(HALF) + 1

    s = 0.125  # 1/sqrt(HD), folded into the q projection
    wq = np.ascontiguousarray((Wq[perm, :] * s).T).astype(BF16)
    wk = np.ascontiguousarray(Wk[perm, :].T).astype(BF16)
    wv = np.ascontiguousarray(Wv[hs, :].T).astype(BF16)

    p32 = np.arange(128) % 32
    cs2 = cos[:, p32].T.astype(BF16)
    sgn = np.where((np.arange(128) // 32) % 2 == 0, -1.0, 1.0).astype(np.float32)
    sn2 = (sin[:, p32] * sgn[None, :]).T.astype(BF16)

    kk = np.arange(128)[:, None]
    qq = np.arange(128)[None, :]
    tri = (kk <= qq)  # [128, 128] triangle for the diagonal tiles
    msk = np.concatenate([tri, tri], axis=1).astype(BF16)  # [128, 256]

    return {
        "xq": np.ascontiguousarray(q[b].T).astype(BF16),
        "xk": np.ascontiguousarray(k[b].T).astype(BF16),
        "xv": np.ascontiguousarray(v[b].T).astype(BF16),
        "wq": wq, "wk": wk, "wv": wv,
        "bqc": np.ascontiguousarray((bq[perm] * s).reshape(NT, 128).T, np.float32),
        "bkc": np.ascontiguousarray(bk[perm].reshape(NT, 128).T, np.float32),
        "cs2": cs2, "sn2": sn2, "msk": msk,
    }


def prep_in_maps(q, k, v, Wq, bq, Wk, bk, Wv, bv, sin, cos):
    args = [np.asarray(a, np.float32) for a in (q, k, v, Wq, bq, Wk, bk, Wv, bv, sin, cos)]
    maps = [_prep_core_inputs(c, *args) for c in range(8)]
    return maps, args[8]  # bv needed on host in assemble()


def assemble(results, bv):
    out = np.empty((BATCH, SEQ, EMB), np.float32)
    for c in range(8):
        b, hh = c // 2, c % 2
        outT = np.asarray(results[c]["outT"], np.float32)
        l = np.asarray(results[c]["lsum"], np.float32)
        a = outT.reshape(HPC, HD, SEQ) / l[:, None, :]
        out[b, :, hh * DH:(hh + 1) * DH] = a.reshape(DH, SEQ).T \
            + bv[hh * DH:(hh + 1) * DH][None, :]
    return out


def run(in_maps, causal=True, trace=False, **kw):
    _install_ntff_shim()
    from concourse.bass_utils import run_bass_kernel_spmd
    nc = build(causal)
    return run_bass_kernel_spmd(nc, in_maps, core_ids=list(range(8)), trace=trace, **kw)


def kernel(q, k, v, Wq, bq, Wk, bk, Wv, bv, sin, cos, mask):
    in_maps, bv_f = prep_in_maps(q, k, v, Wq, bq, Wk, bk, Wv, bv, sin, cos)
    r = run(in_maps, causal=bool(mask))
    return assemble(r.results, bv_f)


# revision 20
# speedup vs baseline: 1.0272x; 1.0179x over previous
"""Multi-head causal attention (QKV proj + RoPE + softmax) on 8 TRN2 NeuronCores.

Sharding: batch 4-way x head-group 2-way -> each core handles 1 batch and 8
contiguous heads (512 output channels). No collectives; host gathers slices.

v2 schedule: the projections are no longer a serial prelude. Attention units
start as soon as qh[m0, 0:512], kh[m0, 0:512] and vsb[st0] exist, and the
remaining projection work is drip-fed into the PE queue between unit k-tiles,
filling PE slack in the Act-gated softmax pipeline (exp of [128,1024] =
~1.1us vs ~0.65us PE per k-tile). The unit stream is software-pipelined one
k-tile deep (scores of step i+1 issue before PV of step i) so the in-order
PE queue never head-blocks on an exp. x tiles are DMA'd quarter-wise in
consumption order via prefetch pseudo-chains (bounded SBUF pool, no Sync
queue deadlock); RoPE swaps, output DMAs and causal-mask multiplies run on
the otherwise idle GpSimd queue.

Per-core algorithm (matmul compute in bf16, fp32 PSUM accumulation):
  - host passes x.T and W.T shards so matmuls contract over the partition dim.
  - q/k weights are row-permuted per head into [even|odd] so RoPE becomes
    rot = x*cs + swap32(x)*sn (swap32 = SBUF partition-block swap by DMA).
  - scores are computed transposed, S_T[k, q] = kh_T.T @ qh_T, with the A/B
    heads of a 128-row tile on PE row groups 0/1 running concurrently.
  - causal diagonal tiles are NARROWED: scores/exp/PV touch only the
    unmasked q-range (q >= 128*p within the tile); the remaining 128-wide
    triangle is masked by one GpSimd multiply.
  - attnT[d, q] = sum_kt V_tile[k,d|1].T @ P_T[k, q]; a ones-column in V
    makes row 64 the softmax denominator. Division + transpose + v-bias on
    host.
"""

import sys
import types

import numpy as np
import ml_dtypes

BF16 = ml_dtypes.bfloat16
SEQ, EMB, NHEADS, BATCH = 2048, 1024, 16, 4
HD, HALF = 64, 32
HPC = 8          # heads per core
DH = 512         # output dims per core
NE = EMB // 128  # 8 contraction tiles
NT = 4           # head-pair (128-row) dout tiles
NKT = SEQ // 128  # 16 key tiles
NQC = SEQ // 512  # 4 query chunks


def _install_ntff_shim():
    """The image's antenv lacks axon_hooks; synthesize it from trn_agent_boot
    so run_bass_kernel_spmd(trace=True) can profile. Harmless if unused."""
    try:
        import antenv.axon_hooks  # noqa: F401
        return
    except ImportError:
        pass
    try:
        from trn_agent_boot.trn_boot import _ntff_profile_via_ctypes
        import antenv
    except ImportError:
        return
    hook = _ntff_profile_via_ctypes("/opt/axon/libaxon_pjrt.so")
    mod = types.ModuleType("antenv.axon_hooks")
    mod.get_axon_ntff_profile_hook = lambda: hook
    mod.set_axon_ntff_profile_hook = lambda h: None
    sys.modules["antenv.axon_hooks"] = mod
    antenv.axon_hooks = mod


_built = {}


def build(causal=True):
    if causal in _built:
        return _built[causal]
    import concourse.mybir as mybir
    import concourse.tile as tile
    from concourse import bacc

    f32 = mybir.dt.float32
    bf = mybir.dt.bfloat16
    EXP = mybir.ActivationFunctionType.Exp
    MUL = mybir.AluOpType.mult
    ADD = mybir.AluOpType.add

    nc = bacc.Bacc(None, target_bir_lowering=False, debug=False)
    with tile.TileContext(nc) as tc:
        with tc.tile_pool(name="dram", bufs=1, space="DRAM") as dram:
            xq_d = dram.tile([128, NQC, NE * 512], bf, kind="ExternalInput", name="xq", uniquify=False)
            xk_d = dram.tile([128, NQC, NE * 512], bf, kind="ExternalInput", name="xk", uniquify=False)
            xv_d = dram.tile([128, NQC, NE * 512], bf, kind="ExternalInput", name="xv", uniquify=False)
            wq_d = dram.tile([128, NE * DH], bf, kind="ExternalInput", name="wq", uniquify=False)
            wk_d = dram.tile([128, NE * DH], bf, kind="ExternalInput", name="wk", uniquify=False)
            wv_d = dram.tile([128, NE * DH], bf, kind="ExternalInput", name="wv", uniquify=False)
            bqc_d = dram.tile([128, NT], f32, kind="ExternalInput", name="bqc", uniquify=False)
            bkc_d = dram.tile([128, NT], f32, kind="ExternalInput", name="bkc", uniquify=False)
            cs_d = dram.tile([128, SEQ], bf, kind="ExternalInput", name="cs2", uniquify=False)
            sn_d = dram.tile([128, SEQ], bf, kind="ExternalInput", name="sn2", uniquify=False)
            mk_d = dram.tile([128, 256], bf, kind="ExternalInput", name="msk", uniquify=False)
            outT_d = dram.tile([DH, SEQ], f32, kind="ExternalOutput", name="outT", uniquify=False)
            l_d = dram.tile([HPC, SEQ], f32, kind="ExternalOutput", name="lsum", uniquify=False)

            with tc.tile_pool(name="const", bufs=1) as cp, \
                 tc.tile_pool(name="xin", bufs=8) as xp, \
                 tc.tile_pool(name="rope", bufs=3) as rp, \
                 tc.tile_pool(name="ostage", bufs=4) as op, \
                 tc.tile_pool(name="pp", bufs=2, space="PSUM") as pp, \
                 tc.tile_pool(name="sp", bufs=2, space="PSUM") as sp, \
                 tc.tile_pool(name="tA", bufs=1, space="PSUM") as ptA, \
                 tc.tile_pool(name="tB", bufs=1, space="PSUM") as ptB:

                qh = cp.tile([128, NT, SEQ], bf, name="qh")
                kh = cp.tile([128, NT, SEQ], bf, name="kh")
                vsb = cp.tile([128, NKT, HPC * 65], bf, name="vsb")
                probs = cp.tile([128, 2, NKT, 512], bf, name="probs")
                w_sb = {n: cp.tile([128, NE, DH], bf, name=f"w_{n}") for n in "qkv"}
                b_sb = {n: cp.tile([128, NT], f32, name=f"b_{n}") for n in "qk"}
                cs = cp.tile([128, SEQ], bf, name="cs")
                sn = cp.tile([128, SEQ], bf, name="sn")
                msk = cp.tile([128, 2, 128], bf, name="mskt")

                # small constants first
                nc.sync.dma_start(out=b_sb["q"][:, :], in_=bqc_d[:, :])
                nc.sync.dma_start(out=b_sb["k"][:, :], in_=bkc_d[:, :])
                nc.sync.dma_start(out=cs[:, :], in_=cs_d[:, :])
                nc.sync.dma_start(out=sn[:, :], in_=sn_d[:, :])
                nc.sync.dma_start(out=msk[:, :, :],
                                  in_=mk_d[:, :].rearrange("p (h u) -> p h u", h=2))
                nc.vector.memset(
                    vsb[:, :, :].rearrange("p k (h u) -> p k h u", u=65)[:, :, :, 64:65],
                    1.0)

                xt = {}      # (nm, c4) -> [128, NE, 512] SBUF tile
                wd = {"q": wq_d, "k": wk_d, "v": wv_d}
                xd = {"q": xq_d, "k": xk_d, "v": xv_d}

                def chain_dma(nm, c4):
                    """Prefetch pseudo-chain: one 3D DMA stages the whole x
                    quarter (+ the weight matrix, on the first quarter)."""
                    if c4 == 0:
                        nc.sync.dma_start(
                            out=w_sb[nm][:, :, :],
                            in_=wd[nm][:, :].rearrange("p (e d) -> p e d", e=NE))
                    t = xp.tile([128, NE, 512], bf, tag="x", name=f"x{nm}_{c4}")
                    nc.sync.dma_start(
                        out=t[:, :, :],
                        in_=xd[nm][:, c4, :].rearrange("p (e s) -> p e s", e=NE))
                    xt[(nm, c4)] = t
                    return
                    yield  # pragma: no cover - make this a generator

                # ---- projection chains (generators; each yield ~= 2 e-pairs
                # of PE work). qk chain: one (m, q-chunk) -> psum -> bias-add
                # eviction -> partition-swap DMA -> RoPE into qh/kh. ----
                def chain_qk(nm, m, c):
                    dst = qh if nm == "q" else kh
                    ps = pp.tile([128, 512], f32, tag="p", name=f"pp{nm}{m}{c}")
                    for e in range(NE):
                        nc.tensor.matmul(
                            ps[0:64, :],
                            w_sb[nm][:, e, m * 128:m * 128 + 64],
                            xt[(nm, c)][:, e, :],
                            start=(e == 0), stop=(e == NE - 1),
                            skip_group_check=True)
                        nc.tensor.matmul(
                            ps[64:128, :],
                            w_sb[nm][:, e, m * 128 + 64:(m + 1) * 128],
                            xt[(nm, c)][:, e, :],
                            start=(e == 0), stop=(e == NE - 1),
                            skip_group_check=True)
                        if e % 2 == 1 and e < NE - 1:
                            yield
                    tmp = rp.tile([128, 512], bf, tag="tmp", bufs=6, name=f"t{nm}{m}{c}")
                    nc.vector.tensor_scalar_add(tmp[:, :], ps[:, :],
                                                b_sb[nm][:, m:m + 1])
                    tsw = rp.tile([128, 512], bf, tag="tsw", bufs=4, name=f"w{nm}{m}{c}")
                    for blk in range(4):
                        s = blk ^ 1
                        nc.gpsimd.dma_start(out=tsw[blk * 32:(blk + 1) * 32, :],
                                            in_=tmp[s * 32:(s + 1) * 32, :])
                    yield
                    cc = slice(c * 512, (c + 1) * 512)
                    m2 = rp.tile([128, 512], bf, tag="m2", bufs=4, name=f"m{nm}{m}{c}")
                    nc.vector.tensor_tensor(dst[:, m, cc], tmp[:, :], cs[:, cc], MUL)
                    nc.vector.tensor_tensor(m2[:, :], tsw[:, :], sn[:, cc], MUL)
                    nc.vector.tensor_tensor(dst[:, m, cc], dst[:, m, cc], m2[:, :], ADD)

                def chain_v(st):
                    c4, o = st // 4, (st % 4) * 128
                    ps = pp.tile([128, 512], f32, tag="p", name=f"ppv{st}")
                    for e in range(NE):
                        nc.tensor.matmul(
                            ps[0:64, :],
                            xt[("v", c4)][:, e, o:o + 64],
                            w_sb["v"][:, e, :],
                            start=(e == 0), stop=(e == NE - 1),
                            skip_group_check=True)
                        nc.tensor.matmul(
                            ps[64:128, :],
                            xt[("v", c4)][:, e, o + 64:o + 128],
                            w_sb["v"][:, e, :],
                            start=(e == 0), stop=(e == NE - 1),
                            skip_group_check=True)
                        if e % 2 == 1 and e < NE - 1:
                            yield
                    nc.vector.tensor_copy(
                        vsb[:, st, :]
                        .rearrange("p (h u) -> p h u", u=65)[:, :, 0:64],
                        ps[:, :].rearrange("p (h d) -> p h d", d=64))

                # unit emission order and the matching chain priority order
                unit_order = [(0, 0), (1, 0), (0, 1), (2, 0), (1, 1), (3, 0),
                              (2, 1), (0, 2), (3, 1), (1, 2), (2, 2), (0, 3),
                              (3, 2), (1, 3), (2, 3), (3, 3)]
                spec_list = [
                    ("D", "q", 0), ("Q", 0, 0), ("D", "k", 0), ("K", 0, 0),
                    ("D", "v", 0), ("V", 0), ("V", 1), ("V", 2), ("V", 3),
                    ("Q", 1, 0), ("K", 1, 0),
                    ("D", "q", 1), ("Q", 0, 1), ("D", "k", 1), ("K", 0, 1),
                    ("D", "v", 1), ("V", 4), ("V", 5), ("V", 6), ("V", 7),
                    ("Q", 2, 0), ("K", 2, 0), ("Q", 1, 1), ("K", 1, 1),
                    ("Q", 3, 0), ("K", 3, 0), ("Q", 2, 1), ("K", 2, 1),
                    ("D", "q", 2), ("Q", 0, 2), ("D", "k", 2), ("K", 0, 2),
                    ("D", "v", 2), ("V", 8), ("V", 9), ("V", 10), ("V", 11),
                    ("Q", 3, 1), ("K", 3, 1), ("Q", 1, 2), ("K", 1, 2),
                    ("Q", 2, 2), ("K", 2, 2),
                    ("D", "q", 3), ("Q", 0, 3), ("D", "k", 3), ("K", 0, 3),
                    ("D", "v", 3), ("V", 12), ("V", 13), ("V", 14), ("V", 15),
                    ("Q", 3, 2), ("K", 3, 2), ("Q", 1, 3), ("K", 1, 3),
                    ("Q", 2, 3), ("K", 2, 3), ("Q", 3, 3), ("K", 3, 3),
                ]
                gens = []
                ord_of = {}
                for spec in spec_list:
                    ord_of[spec] = len(gens)
                    if spec[0] == "D":
                        gens.append(chain_dma(spec[1], spec[2]))
                    elif spec[0] == "V":
                        gens.append(chain_v(spec[1]))
                    else:
                        gens.append(chain_qk(spec[0].lower(), spec[1], spec[2]))
                state = {"pos": 0}

                def drive(n):
                    while n > 0 and state["pos"] < len(gens):
                        try:
                            next(gens[state["pos"]])
                            n -= 1
                        except StopIteration:
                            state["pos"] += 1

                def require(spec):
                    """Fully emit every chain up to and including `spec` —
                    Tile dependencies are tracked in EMISSION order, so a
                    consumer must never be emitted before its producer."""
                    target = ord_of[spec] + 1
                    while state["pos"] < target:
                        try:
                            next(gens[state["pos"]])
                        except StopIteration:
                            state["pos"] += 1

                # ---- flat, 1-deep software-pipelined unit stream ----
                def nkt_of(j):
                    return 4 * (j + 1) if causal else NKT

                seq_steps = [(t, j, kt) for (t, j) in unit_order
                             for kt in range(nkt_of(j))]
                pt_cur = {}
                ps_of = {}

                def emit_scores_exp(t, j, kt):
                    require(("Q", t, j))
                    require(("K", t, kt // 4))
                    p = kt - 4 * j
                    q0 = 128 * p if (causal and p > 0) else 0
                    ps = sp.tile([128, 1024], f32, tag="s", name=f"ps{t}{j}_{kt}")
                    ps_of[(t, j, kt)] = (ps, q0)
                    for half in (0, 1):
                        po = half * 64
                        nc.tensor.matmul(
                            ps[:, half * 512 + q0:(half + 1) * 512],
                            kh[po:po + 64, t, kt * 128:(kt + 1) * 128],
                            qh[po:po + 64, t, j * 512 + q0:(j + 1) * 512],
                            start=True, stop=True, skip_group_check=True)
                    nc.scalar.activation(
                        probs[:, :, kt, q0:512],
                        ps[:, :].rearrange("p (h u) -> p h u", h=2)[:, :, q0:512],
                        EXP)
                    if causal and p >= 0:
                        nc.gpsimd.tensor_tensor(
                            probs[:, :, kt, q0:q0 + 128],
                            probs[:, :, kt, q0:q0 + 128],
                            msk[:, :, :], MUL)

                def emit_pv(t, j, kt):
                    require(("V", kt))
                    nkt = nkt_of(j)
                    if kt == 0:
                        pt_cur[0] = ptA.tile([65, 512], f32, tag="t0", name=f"pt0_{t}{j}")
                        pt_cur[1] = ptB.tile([65, 512], f32, tag="t1", name=f"pt1_{t}{j}")
                    _, q0 = ps_of.pop((t, j, kt))
                    for half in (0, 1):
                        lh = 2 * t + half
                        nc.tensor.matmul(
                            pt_cur[half][:, q0:512],
                            vsb[:, kt, lh * 65:(lh + 1) * 65],
                            probs[:, half, kt, q0:512],
                            start=(kt == 0), stop=(kt == nkt - 1),
                            skip_group_check=True)
                    if kt == nkt - 1:
                        for half in (0, 1):
                            lh = 2 * t + half
                            ost = op.tile([65, 512], f32, tag="ost",
                                          name=f"os{half}_{t}{j}")
                            nc.vector.tensor_copy(ost[:, :], pt_cur[half][:, :])
                            nc.sync.dma_start(
                                out=outT_d[lh * 64:(lh + 1) * 64,
                                           j * 512:(j + 1) * 512],
                                in_=ost[0:64, :])
                            nc.sync.dma_start(
                                out=l_d[lh:lh + 1, j * 512:(j + 1) * 512],
                                in_=ost[64:65, :])

                # prelude: D(q0), Q00, D(k0), K00, D(v0), V0
                require(("V", 0))
                n = len(seq_steps)
                for i in range(n + 1):
                    if i < n:
                        emit_scores_exp(*seq_steps[i])
                        drive(1)
                    if i > 0:
                        emit_pv(*seq_steps[i - 1])
                        drive(1)
                while state["pos"] < len(gens):
                    drive(8)
    _built[causal] = nc
    nc.compile()
    return nc


def _prep_core_inputs(c, q, k, v, Wq, bq, Wk, bk, Wv, bv, sin, cos):
    b, hh = c // 2, c % 2
    hs = slice(hh * DH, (hh + 1) * DH)

    perm = np.empty(DH, np.int64)
    for lh in range(HPC):
        base = (hh * HPC + lh) * HD
        perm[lh * HD:lh * HD + HALF] = base + 2 * np.arange(HALF)
        perm[lh * HD + HALF:(lh + 1) * HD] = base + 2 * np.arange(HALF) + 1

    s = 0.125  # 1/sqrt(HD), folded into the q projection
    wq = np.ascontiguousarray((Wq[perm, :] * s).T).astype(BF16)
    wk = np.ascontiguousarray(Wk[perm, :].T).astype(BF16)
    wv = np.ascontiguousarray(Wv[hs, :].T).astype(BF16)

    p32 = np.arange(128) % 32
    cs2 = cos[:, p32].T.astype(BF16)
    sgn = np.where((np.arange(128) // 32) % 2 == 0, -1.0, 1.0).astype(np.float32)
    sn2 = (sin[:, p32] * sgn[None, :]).T.astype(BF16)

    kk = np.arange(128)[:, None]
    qq = np.arange(128)[None, :]
    tri = (kk <= qq)  # [128, 128] triangle for the diagonal tiles
    msk = np.concatenate([tri, tri], axis=1).astype(BF16)  # [128, 256]

    def xprep(x):
        # [seq, emb] -> x.T [emb, seq] -> [p, c, e*512+s] matching SBUF layout
        xT = np.ascontiguousarray(x.T).astype(BF16)
        return np.ascontiguousarray(
            xT.reshape(NE, 128, NQC, 512).transpose(1, 2, 0, 3)
            .reshape(128, NQC, NE * 512))

    def wprep(w):
        # [emb, dh] -> [p, e*dh] matching SBUF layout
        return np.ascontiguousarray(
            w.reshape(NE, 128, DH).transpose(1, 0, 2).reshape(128, NE * DH))

    return {
        "xq": xprep(q[b]), "xk": xprep(k[b]), "xv": xprep(v[b]),
        "wq": wprep(wq), "wk": wprep(wk), "wv": wprep(wv),
        "bqc": np.ascontiguousarray((bq[perm] * s).reshape(NT, 128).T, np.float32),
        "bkc": np.ascontiguousarray(bk[perm].reshape(NT, 128).T, np.float32),
        "cs2": cs2, "sn2": sn2, "msk": msk,
    }


def prep_in_maps(q, k, v, Wq, bq, Wk, bk, Wv, bv, sin, cos):
    args = [np.asarray(a, np.float32) for a in (q, k, v, Wq, bq, Wk, bk, Wv, bv, sin, cos)]
    maps = [_prep_core_inputs(c, *args) for c in range(8)]
    return maps, args[8]  # bv needed on host in assemble()


def assemble(results, bv):
    out = np.empty((BATCH, SEQ, EMB), np.float32)
    for c in range(8):
        b, hh = c // 2, c % 2
        outT = np.asarray(results[c]["outT"], np.float32)
        l = np.asarray(results[c]["lsum"], np.float32)
        a = outT.reshape(HPC, HD, SEQ) / l[:, None, :]
        out[b, :, hh * DH:(hh + 1) * DH] = a.reshape(DH, SEQ).T \
            + bv[hh * DH:(hh + 1) * DH][None, :]
    return out


def run(in_maps, causal=True, trace=False, **kw):
    _install_ntff_shim()
    from concourse.bass_utils import run_bass_kernel_spmd
    nc = build(causal)
    return run_bass_kernel_spmd(nc, in_maps, core_ids=list(range(8)), trace=trace, **kw)


def kernel(q, k, v, Wq, bq, Wk, bk, Wv, bv, sin, cos, mask):
    in_maps, bv_f = prep_in_maps(q, k, v, Wq, bq, Wk, bk, Wv, bv, sin, cos)
    r = run(in_maps, causal=bool(mask))
    return assemble(r.results, bv_f)


# revision 21
# speedup vs baseline: 1.0863x; 1.0576x over previous
"""Multi-head causal attention (QKV proj + RoPE + softmax) on 8 TRN2 NeuronCores.

Sharding: batch 4-way x head-group 2-way -> each core handles 1 batch and 8
contiguous heads (512 output channels). No collectives; host gathers slices.

v2 schedule: the projections are no longer a serial prelude. Attention units
start as soon as qh[m0, 0:512], kh[m0, 0:512] and vsb[st0] exist, and the
remaining projection work is drip-fed into the PE queue between unit k-tiles,
filling PE slack in the Act-gated softmax pipeline (exp of [128,1024] =
~1.1us vs ~0.65us PE per k-tile). The unit stream is software-pipelined one
k-tile deep (scores of step i+1 issue before PV of step i) so the in-order
PE queue never head-blocks on an exp. x tiles are DMA'd quarter-wise in
consumption order via prefetch pseudo-chains (bounded SBUF pool, no Sync
queue deadlock); RoPE swaps, output DMAs and causal-mask multiplies run on
the otherwise idle GpSimd queue.

Per-core algorithm (matmul compute in bf16, fp32 PSUM accumulation):
  - host passes x.T and W.T shards so matmuls contract over the partition dim.
  - q/k weights are row-permuted per head into [even|odd] so RoPE becomes
    rot = x*cs + swap32(x)*sn (swap32 = SBUF partition-block swap by DMA).
  - scores are computed transposed, S_T[k, q] = kh_T.T @ qh_T, with the A/B
    heads of a 128-row tile on PE row groups 0/1 running concurrently.
  - causal diagonal tiles are NARROWED: scores/exp/PV touch only the
    unmasked q-range (q >= 128*p within the tile); the remaining 128-wide
    triangle is masked by one GpSimd multiply.
  - attnT[d, q] = sum_kt V_tile[k,d|1].T @ P_T[k, q]; a ones-column in V
    makes row 64 the softmax denominator. Division + transpose + v-bias on
    host.
"""

import sys
import types

import numpy as np
import ml_dtypes

BF16 = ml_dtypes.bfloat16
SEQ, EMB, NHEADS, BATCH = 2048, 1024, 16, 4
HD, HALF = 64, 32
HPC = 8          # heads per core
DH = 512         # output dims per core
NE = EMB // 128  # 8 contraction tiles
NT = 4           # head-pair (128-row) dout tiles
NKT = SEQ // 128  # 16 key tiles
NQC = SEQ // 512  # 4 query chunks


def _install_ntff_shim():
    """The image's antenv lacks axon_hooks; synthesize it from trn_agent_boot
    so run_bass_kernel_spmd(trace=True) can profile. Harmless if unused."""
    try:
        import antenv.axon_hooks  # noqa: F401
        return
    except ImportError:
        pass
    try:
        from trn_agent_boot.trn_boot import _ntff_profile_via_ctypes
        import antenv
    except ImportError:
        return
    hook = _ntff_profile_via_ctypes("/opt/axon/libaxon_pjrt.so")
    mod = types.ModuleType("antenv.axon_hooks")
    mod.get_axon_ntff_profile_hook = lambda: hook
    mod.set_axon_ntff_profile_hook = lambda h: None
    sys.modules["antenv.axon_hooks"] = mod
    antenv.axon_hooks = mod


_built = {}


def build(causal=True):
    if causal in _built:
        return _built[causal]
    import concourse.mybir as mybir
    import concourse.tile as tile
    from concourse import bacc

    f32 = mybir.dt.float32
    bf = mybir.dt.bfloat16
    EXP = mybir.ActivationFunctionType.Exp
    MUL = mybir.AluOpType.mult
    ADD = mybir.AluOpType.add

    nc = bacc.Bacc(None, target_bir_lowering=False, debug=False)
    with tile.TileContext(nc) as tc:
        with tc.tile_pool(name="dram", bufs=1, space="DRAM") as dram:
            xq_d = dram.tile([128, NQC, NE * 512], bf, kind="ExternalInput", name="xq", uniquify=False)
            xk_d = dram.tile([128, NQC, NE * 512], bf, kind="ExternalInput", name="xk", uniquify=False)
            xv_d = dram.tile([128, NQC, NE * 512], bf, kind="ExternalInput", name="xv", uniquify=False)
            wq_d = dram.tile([128, NE * DH], bf, kind="ExternalInput", name="wq", uniquify=False)
            wk_d = dram.tile([128, NE * DH], bf, kind="ExternalInput", name="wk", uniquify=False)
            wv_d = dram.tile([128, NE * DH], bf, kind="ExternalInput", name="wv", uniquify=False)
            bqc_d = dram.tile([128, NT], f32, kind="ExternalInput", name="bqc", uniquify=False)
            bkc_d = dram.tile([128, NT], f32, kind="ExternalInput", name="bkc", uniquify=False)
            cs_d = dram.tile([128, SEQ], bf, kind="ExternalInput", name="cs2", uniquify=False)
            sn_d = dram.tile([128, SEQ], bf, kind="ExternalInput", name="sn2", uniquify=False)
            mk_d = dram.tile([128, 256], bf, kind="ExternalInput", name="msk", uniquify=False)
            outT_d = dram.tile([DH, SEQ], f32, kind="ExternalOutput", name="outT", uniquify=False)
            l_d = dram.tile([HPC, SEQ], f32, kind="ExternalOutput", name="lsum", uniquify=False)

            with tc.tile_pool(name="const", bufs=1) as cp, \
                 tc.tile_pool(name="xin", bufs=8) as xp, \
                 tc.tile_pool(name="rope", bufs=3) as rp, \
                 tc.tile_pool(name="ostage", bufs=4) as op, \
                 tc.tile_pool(name="pp", bufs=2, space="PSUM") as pp, \
                 tc.tile_pool(name="sp", bufs=2, space="PSUM") as sp, \
                 tc.tile_pool(name="tA", bufs=1, space="PSUM") as ptA, \
                 tc.tile_pool(name="tB", bufs=1, space="PSUM") as ptB:

                qh = cp.tile([128, NT, SEQ], bf, name="qh")
                kh = cp.tile([128, NT, SEQ], bf, name="kh")
                vsb = cp.tile([128, NKT, HPC * 65], bf, name="vsb")
                probs = cp.tile([128, 2, NKT, 512], bf, name="probs")
                w_sb = {n: cp.tile([128, NE, DH], bf, name=f"w_{n}") for n in "qkv"}
                b_sb = {n: cp.tile([128, NT], f32, name=f"b_{n}") for n in "qk"}
                cs = cp.tile([128, SEQ], bf, name="cs")
                sn = cp.tile([128, SEQ], bf, name="sn")
                msk = cp.tile([128, 2, 128], bf, name="mskt")

                # small constants first
                nc.sync.dma_start(out=b_sb["q"][:, :], in_=bqc_d[:, :])
                nc.sync.dma_start(out=b_sb["k"][:, :], in_=bkc_d[:, :])
                nc.sync.dma_start(out=msk[:, :, :],
                                  in_=mk_d[:, :].rearrange("p (h u) -> p h u", h=2))
                nc.vector.memset(
                    vsb[:, :, :].rearrange("p k (h u) -> p k h u", u=65)[:, :, :, 64:65],
                    1.0)

                xt = {}      # (nm, c4) -> [128, NE, 512] SBUF tile
                wd = {"q": wq_d, "k": wk_d, "v": wv_d}
                xd = {"q": xq_d, "k": xk_d, "v": xv_d}

                def chain_dma(nm, c4):
                    """Prefetch pseudo-chain: one 3D DMA stages the whole x
                    quarter (+ the weight matrix, on the first quarter)."""
                    if c4 == 0:
                        wr = wd[nm][:, :].rearrange("p (e d) -> p e d", e=NE)
                        nc.sync.dma_start(out=w_sb[nm][:, 0:4, :], in_=wr[:, 0:4, :])
                        nc.sync.dma_start(out=w_sb[nm][:, 4:8, :], in_=wr[:, 4:8, :])
                    t = xp.tile([128, NE, 512], bf, tag="x", name=f"x{nm}_{c4}")
                    xr = xd[nm][:, c4, :].rearrange("p (e s) -> p e s", e=NE)
                    nc.sync.dma_start(out=t[:, 0:4, :], in_=xr[:, 0:4, :])
                    nc.sync.dma_start(out=t[:, 4:8, :], in_=xr[:, 4:8, :])
                    xt[(nm, c4)] = t
                    if nm == "q" and c4 == 0:
                        nc.sync.dma_start(out=cs[:, :], in_=cs_d[:, :])
                        nc.sync.dma_start(out=sn[:, :], in_=sn_d[:, :])
                    return
                    yield  # pragma: no cover - make this a generator

                # ---- projection chains (generators; each yield ~= 2 e-pairs
                # of PE work). qk chain: one (m, q-chunk) -> psum -> bias-add
                # eviction -> partition-swap DMA -> RoPE into qh/kh. ----
                def chain_qk(nm, m, c):
                    dst = qh if nm == "q" else kh
                    ps = pp.tile([128, 512], f32, tag="p", name=f"pp{nm}{m}{c}")
                    for e in range(NE):
                        nc.tensor.matmul(
                            ps[0:64, :],
                            w_sb[nm][:, e, m * 128:m * 128 + 64],
                            xt[(nm, c)][:, e, :],
                            start=(e == 0), stop=(e == NE - 1),
                            skip_group_check=True)
                        nc.tensor.matmul(
                            ps[64:128, :],
                            w_sb[nm][:, e, m * 128 + 64:(m + 1) * 128],
                            xt[(nm, c)][:, e, :],
                            start=(e == 0), stop=(e == NE - 1),
                            skip_group_check=True)
                        if e % 2 == 1 and e < NE - 1:
                            yield
                    tmp = rp.tile([128, 512], bf, tag="tmp", bufs=6, name=f"t{nm}{m}{c}")
                    nc.vector.tensor_scalar_add(tmp[:, :], ps[:, :],
                                                b_sb[nm][:, m:m + 1])
                    tsw = rp.tile([128, 512], bf, tag="tsw", bufs=4, name=f"w{nm}{m}{c}")
                    for blk in range(4):
                        s = blk ^ 1
                        nc.gpsimd.dma_start(out=tsw[blk * 32:(blk + 1) * 32, :],
                                            in_=tmp[s * 32:(s + 1) * 32, :])
                    yield
                    cc = slice(c * 512, (c + 1) * 512)
                    m2 = rp.tile([128, 512], bf, tag="m2", bufs=4, name=f"m{nm}{m}{c}")
                    nc.vector.tensor_tensor(dst[:, m, cc], tmp[:, :], cs[:, cc], MUL)
                    nc.vector.tensor_tensor(m2[:, :], tsw[:, :], sn[:, cc], MUL)
                    nc.vector.tensor_tensor(dst[:, m, cc], dst[:, m, cc], m2[:, :], ADD)

                def chain_v(st):
                    c4, o = st // 4, (st % 4) * 128
                    ps = pp.tile([128, 512], f32, tag="p", name=f"ppv{st}")
                    for e in range(NE):
                        nc.tensor.matmul(
                            ps[0:64, :],
                            xt[("v", c4)][:, e, o:o + 64],
                            w_sb["v"][:, e, :],
                            start=(e == 0), stop=(e == NE - 1),
                            skip_group_check=True)
                        nc.tensor.matmul(
                            ps[64:128, :],
                            xt[("v", c4)][:, e, o + 64:o + 128],
                            w_sb["v"][:, e, :],
                            start=(e == 0), stop=(e == NE - 1),
                            skip_group_check=True)
                        if e % 2 == 1 and e < NE - 1:
                            yield
                    nc.vector.tensor_copy(
                        vsb[:, st, :]
                        .rearrange("p (h u) -> p h u", u=65)[:, :, 0:64],
                        ps[:, :].rearrange("p (h d) -> p h d", d=64))

                # unit emission order and the matching chain priority order
                unit_order = [(0, 0), (1, 0), (0, 1), (2, 0), (1, 1), (3, 0),
                              (2, 1), (0, 2), (3, 1), (1, 2), (2, 2), (0, 3),
                              (3, 2), (1, 3), (2, 3), (3, 3)]
                spec_list = [
                    ("D", "q", 0), ("Q", 0, 0), ("D", "k", 0), ("K", 0, 0),
                    ("D", "v", 0), ("V", 0), ("V", 1), ("V", 2), ("V", 3),
                    ("Q", 1, 0), ("K", 1, 0),
                    ("D", "q", 1), ("Q", 0, 1), ("D", "k", 1), ("K", 0, 1),
                    ("D", "v", 1), ("V", 4), ("V", 5), ("V", 6), ("V", 7),
                    ("Q", 2, 0), ("K", 2, 0), ("Q", 1, 1), ("K", 1, 1),
                    ("Q", 3, 0), ("K", 3, 0), ("Q", 2, 1), ("K", 2, 1),
                    ("D", "q", 2), ("Q", 0, 2), ("D", "k", 2), ("K", 0, 2),
                    ("D", "v", 2), ("V", 8), ("V", 9), ("V", 10), ("V", 11),
                    ("Q", 3, 1), ("K", 3, 1), ("Q", 1, 2), ("K", 1, 2),
                    ("Q", 2, 2), ("K", 2, 2),
                    ("D", "q", 3), ("Q", 0, 3), ("D", "k", 3), ("K", 0, 3),
                    ("D", "v", 3), ("V", 12), ("V", 13), ("V", 14), ("V", 15),
                    ("Q", 3, 2), ("K", 3, 2), ("Q", 1, 3), ("K", 1, 3),
                    ("Q", 2, 3), ("K", 2, 3), ("Q", 3, 3), ("K", 3, 3),
                ]
                gens = []
                ord_of = {}
                for spec in spec_list:
                    ord_of[spec] = len(gens)
                    if spec[0] == "D":
                        gens.append(chain_dma(spec[1], spec[2]))
                    elif spec[0] == "V":
                        gens.append(chain_v(spec[1]))
                    else:
                        gens.append(chain_qk(spec[0].lower(), spec[1], spec[2]))
                state = {"pos": 0}

                def drive(n):
                    while n > 0 and state["pos"] < len(gens):
                        try:
                            next(gens[state["pos"]])
                            n -= 1
                        except StopIteration:
                            state["pos"] += 1

                def require(spec):
                    """Fully emit every chain up to and including `spec` —
                    Tile dependencies are tracked in EMISSION order, so a
                    consumer must never be emitted before its producer."""
                    target = ord_of[spec] + 1
                    while state["pos"] < target:
                        try:
                            next(gens[state["pos"]])
                        except StopIteration:
                            state["pos"] += 1

                # ---- flat, 1-deep software-pipelined unit stream ----
                def nkt_of(j):
                    return 4 * (j + 1) if causal else NKT

                seq_steps = [(t, j, kt) for (t, j) in unit_order
                             for kt in range(nkt_of(j))]
                pt_cur = {}
                ps_of = {}

                def emit_scores_exp(t, j, kt):
                    require(("Q", t, j))
                    require(("K", t, kt // 4))
                    p = kt - 4 * j
                    q0 = 128 * p if (causal and p > 0) else 0
                    ps = sp.tile([128, 1024], f32, tag="s", name=f"ps{t}{j}_{kt}")
                    ps_of[(t, j, kt)] = (ps, q0)
                    for half in (0, 1):
                        po = half * 64
                        nc.tensor.matmul(
                            ps[:, half * 512 + q0:(half + 1) * 512],
                            kh[po:po + 64, t, kt * 128:(kt + 1) * 128],
                            qh[po:po + 64, t, j * 512 + q0:(j + 1) * 512],
                            start=True, stop=True, skip_group_check=True)
                    nc.scalar.activation(
                        probs[:, :, kt, q0:512],
                        ps[:, :].rearrange("p (h u) -> p h u", h=2)[:, :, q0:512],
                        EXP)
                    if causal and p >= 0:
                        nc.vector.tensor_tensor(
                            probs[:, :, kt, q0:q0 + 128],
                            probs[:, :, kt, q0:q0 + 128],
                            msk[:, :, :], MUL)

                def emit_pv(t, j, kt):
                    require(("V", kt))
                    nkt = nkt_of(j)
                    if kt == 0:
                        pt_cur[0] = ptA.tile([65, 512], f32, tag="t0", name=f"pt0_{t}{j}")
                        pt_cur[1] = ptB.tile([65, 512], f32, tag="t1", name=f"pt1_{t}{j}")
                    _, q0 = ps_of.pop((t, j, kt))
                    for half in (0, 1):
                        lh = 2 * t + half
                        nc.tensor.matmul(
                            pt_cur[half][:, q0:512],
                            vsb[:, kt, lh * 65:(lh + 1) * 65],
                            probs[:, half, kt, q0:512],
                            start=(kt == 0), stop=(kt == nkt - 1),
                            skip_group_check=True)
                    if kt == nkt - 1:
                        for half in (0, 1):
                            lh = 2 * t + half
                            ost = op.tile([65, 512], f32, tag="ost",
                                          name=f"os{half}_{t}{j}")
                            nc.vector.tensor_copy(ost[:, :], pt_cur[half][:, :])
                            nc.sync.dma_start(
                                out=outT_d[lh * 64:(lh + 1) * 64,
                                           j * 512:(j + 1) * 512],
                                in_=ost[0:64, :])
                            nc.sync.dma_start(
                                out=l_d[lh:lh + 1, j * 512:(j + 1) * 512],
                                in_=ost[64:65, :])

                n = len(seq_steps)
                for i in range(n + 1):
                    if i < n:
                        emit_scores_exp(*seq_steps[i])
                        drive(1)
                    if i > 0:
                        emit_pv(*seq_steps[i - 1])
                        drive(1)
                while state["pos"] < len(gens):
                    drive(8)
    _built[causal] = nc
    nc.compile()
    return nc


def _prep_core_inputs(c, q, k, v, Wq, bq, Wk, bk, Wv, bv, sin, cos):
    b, hh = c // 2, c % 2
    hs = slice(hh * DH, (hh + 1) * DH)

    perm = np.empty(DH, np.int64)
    for lh in range(HPC):
        base = (hh * HPC + lh) * HD
        perm[lh * HD:lh * HD + HALF] = base + 2 * np.arange(HALF)
        perm[lh * HD + HALF:(lh + 1) * HD] = base + 2 * np.arange(HALF) + 1

    s = 0.125  # 1/sqrt(HD), folded into the q projection
    wq = np.ascontiguousarray((Wq[perm, :] * s).T).astype(BF16)
    wk = np.ascontiguousarray(Wk[perm, :].T).astype(BF16)
    wv = np.ascontiguousarray(Wv[hs, :].T).astype(BF16)

    p32 = np.arange(128) % 32
    cs2 = cos[:, p32].T.astype(BF16)
    sgn = np.where((np.arange(128) // 32) % 2 == 0, -1.0, 1.0).astype(np.float32)
    sn2 = (sin[:, p32] * sgn[None, :]).T.astype(BF16)

    kk = np.arange(128)[:, None]
    qq = np.arange(128)[None, :]
    tri = (kk <= qq)  # [128, 128] triangle for the diagonal tiles
    msk = np.concatenate([tri, tri], axis=1).astype(BF16)  # [128, 256]

    def xprep(x):
        # [seq, emb] -> x.T [emb, seq] -> [p, c, e*512+s] matching SBUF layout
        xT = np.ascontiguousarray(x.T).astype(BF16)
        return np.ascontiguousarray(
            xT.reshape(NE, 128, NQC, 512).transpose(1, 2, 0, 3)
            .reshape(128, NQC, NE * 512))

    def wprep(w):
        # [emb, dh] -> [p, e*dh] matching SBUF layout
        return np.ascontiguousarray(
            w.reshape(NE, 128, DH).transpose(1, 0, 2).reshape(128, NE * DH))

    return {
        "xq": xprep(q[b]), "xk": xprep(k[b]), "xv": xprep(v[b]),
        "wq": wprep(wq), "wk": wprep(wk), "wv": wprep(wv),
        "bqc": np.ascontiguousarray((bq[perm] * s).reshape(NT, 128).T, np.float32),
        "bkc": np.ascontiguousarray(bk[perm].reshape(NT, 128).T, np.float32),
        "cs2": cs2, "sn2": sn2, "msk": msk,
    }


def prep_in_maps(q, k, v, Wq, bq, Wk, bk, Wv, bv, sin, cos):
    args = [np.asarray(a, np.float32) for a in (q, k, v, Wq, bq, Wk, bk, Wv, bv, sin, cos)]
    maps = [_prep_core_inputs(c, *args) for c in range(8)]
    return maps, args[8]  # bv needed on host in assemble()


def assemble(results, bv):
    out = np.empty((BATCH, SEQ, EMB), np.float32)
    for c in range(8):
        b, hh = c // 2, c % 2
        outT = np.asarray(results[c]["outT"], np.float32)
        l = np.asarray(results[c]["lsum"], np.float32)
        a = outT.reshape(HPC, HD, SEQ) / l[:, None, :]
        out[b, :, hh * DH:(hh + 1) * DH] = a.reshape(DH, SEQ).T \
            + bv[hh * DH:(hh + 1) * DH][None, :]
    return out


def run(in_maps, causal=True, trace=False, **kw):
    _install_ntff_shim()
    from concourse.bass_utils import run_bass_kernel_spmd
    nc = build(causal)
    return run_bass_kernel_spmd(nc, in_maps, core_ids=list(range(8)), trace=trace, **kw)


def kernel(q, k, v, Wq, bq, Wk, bk, Wv, bv, sin, cos, mask):
    in_maps, bv_f = prep_in_maps(q, k, v, Wq, bq, Wk, bk, Wv, bv, sin, cos)
    r = run(in_maps, causal=bool(mask))
    return assemble(r.results, bv_f)


# revision 22
# speedup vs baseline: 1.2597x; 1.1596x over previous
"""Multi-head causal attention (QKV proj + RoPE + softmax) on 8 TRN2 NeuronCores.

Sharding: batch 4-way x head-group 2-way -> each core handles 1 batch and 8
contiguous heads (512 output channels). No collectives; host gathers slices.

v2 schedule: the projections are no longer a serial prelude. Attention units
start as soon as qh[m0, 0:512], kh[m0, 0:512] and vsb[st0] exist, and the
remaining projection work is drip-fed into the PE queue between unit k-tiles,
filling PE slack in the Act-gated softmax pipeline (exp of [128,1024] =
~1.1us vs ~0.65us PE per k-tile). The unit stream is software-pipelined one
k-tile deep (scores of step i+1 issue before PV of step i) so the in-order
PE queue never head-blocks on an exp. x tiles are DMA'd quarter-wise in
consumption order via prefetch pseudo-chains (bounded SBUF pool, no Sync
queue deadlock); RoPE swaps, output DMAs and causal-mask multiplies run on
the otherwise idle GpSimd queue.

Per-core algorithm (matmul compute in bf16, fp32 PSUM accumulation):
  - host passes x.T and W.T shards so matmuls contract over the partition dim.
  - q/k weights are row-permuted per head into [even|odd] so RoPE becomes
    rot = x*cs + swap32(x)*sn (swap32 = SBUF partition-block swap by DMA).
  - scores are computed transposed, S_T[k, q] = kh_T.T @ qh_T, with the A/B
    heads of a 128-row tile on PE row groups 0/1 running concurrently.
  - causal diagonal tiles are NARROWED: scores/exp/PV touch only the
    unmasked q-range (q >= 128*p within the tile); the remaining 128-wide
    triangle is masked by one GpSimd multiply.
  - attnT[d, q] = sum_kt V_tile[k,d|1].T @ P_T[k, q]; a ones-column in V
    makes row 64 the softmax denominator. Division + transpose + v-bias on
    host.
"""

import sys
import types

import numpy as np
import ml_dtypes

BF16 = ml_dtypes.bfloat16
SEQ, EMB, NHEADS, BATCH = 2048, 1024, 16, 4
HD, HALF = 64, 32
HPC = 8          # heads per core
DH = 512         # output dims per core
NE = EMB // 128  # 8 contraction tiles
NT = 4           # head-pair (128-row) dout tiles
NKT = SEQ // 128  # 16 key tiles
NQC = SEQ // 512  # 4 query chunks


def _install_ntff_shim():
    """The image's antenv lacks axon_hooks; synthesize it from trn_agent_boot
    so run_bass_kernel_spmd(trace=True) can profile. Harmless if unused."""
    try:
        import antenv.axon_hooks  # noqa: F401
        return
    except ImportError:
        pass
    try:
        from trn_agent_boot.trn_boot import _ntff_profile_via_ctypes
        import antenv
    except ImportError:
        return
    hook = _ntff_profile_via_ctypes("/opt/axon/libaxon_pjrt.so")
    mod = types.ModuleType("antenv.axon_hooks")
    mod.get_axon_ntff_profile_hook = lambda: hook
    mod.set_axon_ntff_profile_hook = lambda h: None
    sys.modules["antenv.axon_hooks"] = mod
    antenv.axon_hooks = mod


_built = {}


def build(causal=True):
    if causal in _built:
        return _built[causal]
    import concourse.mybir as mybir
    import concourse.tile as tile
    from concourse import bacc

    f32 = mybir.dt.float32
    bf = mybir.dt.bfloat16
    EXP = mybir.ActivationFunctionType.Exp
    MUL = mybir.AluOpType.mult
    ADD = mybir.AluOpType.add

    nc = bacc.Bacc(None, target_bir_lowering=False, debug=False)
    with tile.TileContext(nc) as tc:
        with tc.tile_pool(name="dram", bufs=1, space="DRAM") as dram:
            xq_d = dram.tile([128, NQC, NE * 512], bf, kind="ExternalInput", name="xq", uniquify=False)
            xk_d = dram.tile([128, NQC, NE * 512], bf, kind="ExternalInput", name="xk", uniquify=False)
            xv_d = dram.tile([128, NQC, NE * 512], bf, kind="ExternalInput", name="xv", uniquify=False)
            wq_d = dram.tile([128, NE * DH], bf, kind="ExternalInput", name="wq", uniquify=False)
            wk_d = dram.tile([128, NE * DH], bf, kind="ExternalInput", name="wk", uniquify=False)
            wv_d = dram.tile([128, NE * DH], bf, kind="ExternalInput", name="wv", uniquify=False)
            bqc_d = dram.tile([128, NT], f32, kind="ExternalInput", name="bqc", uniquify=False)
            bkc_d = dram.tile([128, NT], f32, kind="ExternalInput", name="bkc", uniquify=False)
            cs_d = dram.tile([128, SEQ], bf, kind="ExternalInput", name="cs2", uniquify=False)
            sn_d = dram.tile([128, SEQ], bf, kind="ExternalInput", name="sn2", uniquify=False)
            mk_d = dram.tile([128, 256], bf, kind="ExternalInput", name="msk", uniquify=False)
            outT_d = dram.tile([DH, SEQ], f32, kind="ExternalOutput", name="outT", uniquify=False)
            l_d = dram.tile([HPC, SEQ], f32, kind="ExternalOutput", name="lsum", uniquify=False)

            with tc.tile_pool(name="const", bufs=1) as cp, \
                 tc.tile_pool(name="xin", bufs=8) as xp, \
                 tc.tile_pool(name="rope", bufs=3) as rp, \
                 tc.tile_pool(name="ostage", bufs=4) as op, \
                 tc.tile_pool(name="pp", bufs=2, space="PSUM") as pp, \
                 tc.tile_pool(name="sp", bufs=2, space="PSUM") as sp, \
                 tc.tile_pool(name="tA", bufs=1, space="PSUM") as ptA, \
                 tc.tile_pool(name="tB", bufs=1, space="PSUM") as ptB:

                qh = cp.tile([128, NT, SEQ], bf, name="qh")
                kh = cp.tile([128, NT, SEQ], bf, name="kh")
                vsb = cp.tile([128, NKT, HPC * 65], bf, name="vsb")
                probs = cp.tile([128, 2, NKT, 512], bf, name="probs")
                w_sb = {n: cp.tile([128, NE, DH], bf, name=f"w_{n}") for n in "qkv"}
                b_sb = {n: cp.tile([128, NT], f32, name=f"b_{n}") for n in "qk"}
                cs = cp.tile([128, SEQ], bf, name="cs")
                sn = cp.tile([128, SEQ], bf, name="sn")
                msk = cp.tile([128, 2, 128], bf, name="mskt")

                # small constants first
                nc.sync.dma_start(out=b_sb["q"][:, :], in_=bqc_d[:, :])
                nc.sync.dma_start(out=b_sb["k"][:, :], in_=bkc_d[:, :])
                nc.sync.dma_start(out=msk[:, :, :],
                                  in_=mk_d[:, :].rearrange("p (h u) -> p h u", h=2))
                nc.vector.memset(
                    vsb[:, :, :].rearrange("p k (h u) -> p k h u", u=65)[:, :, :, 64:65],
                    1.0)

                xt = {}      # (nm, c4) -> [128, NE, 512] SBUF tile
                wd = {"q": wq_d, "k": wk_d, "v": wv_d}
                xd = {"q": xq_d, "k": xk_d, "v": xv_d}

                def chain_dma(nm, c4):
                    """Prefetch pseudo-chain: one 3D DMA stages the whole x
                    quarter (+ the weight matrix, on the first quarter)."""
                    if c4 == 0:
                        wr = wd[nm][:, :].rearrange("p (e d) -> p e d", e=NE)
                        nc.sync.dma_start(out=w_sb[nm][:, 0:4, :], in_=wr[:, 0:4, :])
                        nc.sync.dma_start(out=w_sb[nm][:, 4:8, :], in_=wr[:, 4:8, :])
                    t = xp.tile([128, NE, 512], bf, tag="x", name=f"x{nm}_{c4}")
                    xr = xd[nm][:, c4, :].rearrange("p (e s) -> p e s", e=NE)
                    nc.sync.dma_start(out=t[:, 0:4, :], in_=xr[:, 0:4, :])
                    nc.sync.dma_start(out=t[:, 4:8, :], in_=xr[:, 4:8, :])
                    xt[(nm, c4)] = t
                    if nm == "q" and c4 == 0:
                        nc.sync.dma_start(out=cs[:, :], in_=cs_d[:, :])
                        nc.sync.dma_start(out=sn[:, :], in_=sn_d[:, :])
                    return
                    yield  # pragma: no cover - make this a generator

                # ---- projection chains (generators; each yield ~= 2 e-pairs
                # of PE work). qk chain: one (m, q-chunk) -> psum -> bias-add
                # eviction -> partition-swap DMA -> RoPE into qh/kh. ----
                def chain_qk(nm, m, c):
                    dst = qh if nm == "q" else kh
                    ps = pp.tile([128, 512], f32, tag="p", name=f"pp{nm}{m}{c}")
                    for e in range(NE):
                        nc.tensor.matmul(
                            ps[:, :],
                            w_sb[nm][:, e, m * 128:(m + 1) * 128],
                            xt[(nm, c)][:, e, :],
                            start=(e == 0), stop=(e == NE - 1),
                            skip_group_check=True)
                        if e % 2 == 1 and e < NE - 1:
                            yield
                    tmp = rp.tile([128, 512], bf, tag="tmp", bufs=6, name=f"t{nm}{m}{c}")
                    nc.vector.tensor_scalar_add(tmp[:, :], ps[:, :],
                                                b_sb[nm][:, m:m + 1])
                    tsw = rp.tile([128, 512], bf, tag="tsw", bufs=4, name=f"w{nm}{m}{c}")
                    for blk in range(4):
                        s = blk ^ 1
                        nc.gpsimd.dma_start(out=tsw[blk * 32:(blk + 1) * 32, :],
                                            in_=tmp[s * 32:(s + 1) * 32, :])
                    yield
                    cc = slice(c * 512, (c + 1) * 512)
                    m2 = rp.tile([128, 512], bf, tag="m2", bufs=4, name=f"m{nm}{m}{c}")
                    nc.vector.tensor_tensor(dst[:, m, cc], tmp[:, :], cs[:, cc], MUL)
                    nc.vector.tensor_tensor(m2[:, :], tsw[:, :], sn[:, cc], MUL)
                    nc.vector.tensor_tensor(dst[:, m, cc], dst[:, m, cc], m2[:, :], ADD)

                def chain_v(st):
                    c4, o = st // 4, (st % 4) * 128
                    ps = pp.tile([128, 512], f32, tag="p", name=f"ppv{st}")
                    for e in range(NE):
                        nc.tensor.matmul(
                            ps[:, :],
                            xt[("v", c4)][:, e, o:o + 128],
                            w_sb["v"][:, e, :],
                            start=(e == 0), stop=(e == NE - 1),
                            skip_group_check=True)
                        if e % 2 == 1 and e < NE - 1:
                            yield
                    nc.vector.tensor_copy(
                        vsb[:, st, :]
                        .rearrange("p (h u) -> p h u", u=65)[:, :, 0:64],
                        ps[:, :].rearrange("p (h d) -> p h d", d=64))

                # unit emission order and the matching chain priority order
                unit_order = [(0, 0), (1, 0), (0, 1), (2, 0), (1, 1), (3, 0),
                              (2, 1), (0, 2), (3, 1), (1, 2), (2, 2), (0, 3),
                              (3, 2), (1, 3), (2, 3), (3, 3)]
                spec_list = [
                    ("D", "q", 0), ("Q", 0, 0), ("D", "k", 0), ("K", 0, 0),
                    ("D", "v", 0), ("V", 0), ("V", 1), ("V", 2), ("V", 3),
                    ("Q", 1, 0), ("K", 1, 0),
                    ("D", "q", 1), ("Q", 0, 1), ("D", "k", 1), ("K", 0, 1),
                    ("D", "v", 1), ("V", 4), ("V", 5), ("V", 6), ("V", 7),
                    ("Q", 2, 0), ("K", 2, 0), ("Q", 1, 1), ("K", 1, 1),
                    ("Q", 3, 0), ("K", 3, 0), ("Q", 2, 1), ("K", 2, 1),
                    ("D", "q", 2), ("Q", 0, 2), ("D", "k", 2), ("K", 0, 2),
                    ("D", "v", 2), ("V", 8), ("V", 9), ("V", 10), ("V", 11),
                    ("Q", 3, 1), ("K", 3, 1), ("Q", 1, 2), ("K", 1, 2),
                    ("Q", 2, 2), ("K", 2, 2),
                    ("D", "q", 3), ("Q", 0, 3), ("D", "k", 3), ("K", 0, 3),
                    ("D", "v", 3), ("V", 12), ("V", 13), ("V", 14), ("V", 15),
                    ("Q", 3, 2), ("K", 3, 2), ("Q", 1, 3), ("K", 1, 3),
                    ("Q", 2, 3), ("K", 2, 3), ("Q", 3, 3), ("K", 3, 3),
                ]
                gens = []
                ord_of = {}
                for spec in spec_list:
                    ord_of[spec] = len(gens)
                    if spec[0] == "D":
                        gens.append(chain_dma(spec[1], spec[2]))
                    elif spec[0] == "V":
                        gens.append(chain_v(spec[1]))
                    else:
                        gens.append(chain_qk(spec[0].lower(), spec[1], spec[2]))
                state = {"pos": 0}

                def drive(n):
                    while n > 0 and state["pos"] < len(gens):
                        try:
                            next(gens[state["pos"]])
                            n -= 1
                        except StopIteration:
                            state["pos"] += 1

                def require(spec):
                    """Fully emit every chain up to and including `spec` —
                    Tile dependencies are tracked in EMISSION order, so a
                    consumer must never be emitted before its producer."""
                    target = ord_of[spec] + 1
                    while state["pos"] < target:
                        try:
                            next(gens[state["pos"]])
                        except StopIteration:
                            state["pos"] += 1

                # ---- flat, 1-deep software-pipelined unit stream ----
                def nkt_of(j):
                    return 4 * (j + 1) if causal else NKT

                seq_steps = [(t, j, kt) for (t, j) in unit_order
                             for kt in range(nkt_of(j))]
                pt_cur = {}
                ps_of = {}

                def emit_scores_exp(t, j, kt):
                    require(("Q", t, j))
                    require(("K", t, kt // 4))
                    p = kt - 4 * j
                    q0 = 128 * p if (causal and p > 0) else 0
                    ps = sp.tile([128, 1024], f32, tag="s", name=f"ps{t}{j}_{kt}")
                    ps_of[(t, j, kt)] = (ps, q0)
                    for half in (0, 1):
                        po = half * 64
                        nc.tensor.matmul(
                            ps[:, half * 512 + q0:(half + 1) * 512],
                            kh[po:po + 64, t, kt * 128:(kt + 1) * 128],
                            qh[po:po + 64, t, j * 512 + q0:(j + 1) * 512],
                            start=True, stop=True, skip_group_check=True)
                    nc.scalar.activation(
                        probs[:, :, kt, q0:512],
                        ps[:, :].rearrange("p (h u) -> p h u", h=2)[:, :, q0:512],
                        EXP)
                    if causal and p >= 0:
                        nc.vector.tensor_tensor(
                            probs[:, :, kt, q0:q0 + 128],
                            probs[:, :, kt, q0:q0 + 128],
                            msk[:, :, :], MUL)

                def emit_pv(t, j, kt):
                    require(("V", kt))
                    nkt = nkt_of(j)
                    if kt == 0:
                        pt_cur[0] = ptA.tile([65, 512], f32, tag="t0", name=f"pt0_{t}{j}")
                        pt_cur[1] = ptB.tile([65, 512], f32, tag="t1", name=f"pt1_{t}{j}")
                    _, q0 = ps_of.pop((t, j, kt))
                    for half in (0, 1):
                        lh = 2 * t + half
                        nc.tensor.matmul(
                            pt_cur[half][:, q0:512],
                            vsb[:, kt, lh * 65:(lh + 1) * 65],
                            probs[:, half, kt, q0:512],
                            start=(kt == 0), stop=(kt == nkt - 1),
                            skip_group_check=True)
                    if kt == nkt - 1:
                        for half in (0, 1):
                            lh = 2 * t + half
                            ost = op.tile([65, 512], f32, tag="ost",
                                          name=f"os{half}_{t}{j}")
                            nc.vector.tensor_copy(ost[:, :], pt_cur[half][:, :])
                            nc.sync.dma_start(
                                out=outT_d[lh * 64:(lh + 1) * 64,
                                           j * 512:(j + 1) * 512],
                                in_=ost[0:64, :])
                            nc.sync.dma_start(
                                out=l_d[lh:lh + 1, j * 512:(j + 1) * 512],
                                in_=ost[64:65, :])

                n = len(seq_steps)
                for i in range(n + 1):
                    if i < n:
                        emit_scores_exp(*seq_steps[i])
                        drive(1)
                    if i > 0:
                        emit_pv(*seq_steps[i - 1])
                        drive(1)
                while state["pos"] < len(gens):
                    drive(8)
    _built[causal] = nc
    nc.compile()
    return nc


def _prep_core_inputs(c, q, k, v, Wq, bq, Wk, bk, Wv, bv, sin, cos):
    b, hh = c // 2, c % 2
    hs = slice(hh * DH, (hh + 1) * DH)

    perm = np.empty(DH, np.int64)
    for lh in range(HPC):
        base = (hh * HPC + lh) * HD
        perm[lh * HD:lh * HD + HALF] = base + 2 * np.arange(HALF)
        perm[lh * HD + HALF:(lh + 1) * HD] = base + 2 * np.arange(HALF) + 1

    s = 0.125  # 1/sqrt(HD), folded into the q projection
    wq = np.ascontiguousarray((Wq[perm, :] * s).T).astype(BF16)
    wk = np.ascontiguousarray(Wk[perm, :].T).astype(BF16)
    wv = np.ascontiguousarray(Wv[hs, :].T).astype(BF16)

    p32 = np.arange(128) % 32
    cs2 = cos[:, p32].T.astype(BF16)
    sgn = np.where((np.arange(128) // 32) % 2 == 0, -1.0, 1.0).astype(np.float32)
    sn2 = (sin[:, p32] * sgn[None, :]).T.astype(BF16)

    kk = np.arange(128)[:, None]
    qq = np.arange(128)[None, :]
    tri = (kk <= qq)  # [128, 128] triangle for the diagonal tiles
    msk = np.concatenate([tri, tri], axis=1).astype(BF16)  # [128, 256]

    def xprep(x):
        # [seq, emb] -> x.T [emb, seq] -> [p, c, e*512+s] matching SBUF layout
        xT = np.ascontiguousarray(x.T).astype(BF16)
        return np.ascontiguousarray(
            xT.reshape(NE, 128, NQC, 512).transpose(1, 2, 0, 3)
            .reshape(128, NQC, NE * 512))

    def wprep(w):
        # [emb, dh] -> [p, e*dh] matching SBUF layout
        return np.ascontiguousarray(
            w.reshape(NE, 128, DH).transpose(1, 0, 2).reshape(128, NE * DH))

    return {
        "xq": xprep(q[b]), "xk": xprep(k[b]), "xv": xprep(v[b]),
        "wq": wprep(wq), "wk": wprep(wk), "wv": wprep(wv),
        "bqc": np.ascontiguousarray((bq[perm] * s).reshape(NT, 128).T, np.float32),
        "bkc": np.ascontiguousarray(bk[perm].reshape(NT, 128).T, np.float32),
        "cs2": cs2, "sn2": sn2, "msk": msk,
    }


def prep_in_maps(q, k, v, Wq, bq, Wk, bk, Wv, bv, sin, cos):
    args = [np.asarray(a, np.float32) for a in (q, k, v, Wq, bq, Wk, bk, Wv, bv, sin, cos)]
    maps = [_prep_core_inputs(c, *args) for c in range(8)]
    return maps, args[8]  # bv needed on host in assemble()


def assemble(results, bv):
    out = np.empty((BATCH, SEQ, EMB), np.float32)
    for c in range(8):
        b, hh = c // 2, c % 2
        outT = np.asarray(results[c]["outT"], np.float32)
        l = np.asarray(results[c]["lsum"], np.float32)
        a = outT.reshape(HPC, HD, SEQ) / l[:, None, :]
        out[b, :, hh * DH:(hh + 1) * DH] = a.reshape(DH, SEQ).T \
            + bv[hh * DH:(hh + 1) * DH][None, :]
    return out


def run(in_maps, causal=True, trace=False, **kw):
    _install_ntff_shim()
    from concourse.bass_utils import run_bass_kernel_spmd
    nc = build(causal)
    return run_bass_kernel_spmd(nc, in_maps, core_ids=list(range(8)), trace=trace, **kw)


def kernel(q, k, v, Wq, bq, Wk, bk, Wv, bv, sin, cos, mask):
    in_maps, bv_f = prep_in_maps(q, k, v, Wq, bq, Wk, bk, Wv, bv, sin, cos)
    r = run(in_maps, causal=bool(mask))
    return assemble(r.results, bv_f)
